# revision 1
# baseline (speedup 1.0000x reference)
"""Trainium2 Bass kernel for gated single-head attention (B=4, L=2048, E=512, D=64).

Sharding: data-parallel over 8 cores; core c handles batch b=c//2, query-row
half h=c%2 (1024 query rows). k/v are computed per-core for the full batch
(duplicated across the pair) since softmax needs all 2048 keys.

Math restructuring (validated in numpy + CoreSim against the jax reference):
  - q,k are L2-normalized so scores lie in [-1/8, 1/8]: softmax needs no
    max-subtraction; exp() applies directly to the transposed score tile,
    with 1/||k_j|| folded into the Exp per-partition scale operand.
  - softmax denominator Z is skipped entirely: rmsnorm is invariant to a
    per-row uniform scale (only the eps semantics shift, ~1e-5 effect).
  - all gates go through Tanh (sigma(x) = tanh(x/2)/2 + 1/2, the 1/2s folded
    into host-side weights) so the only ACT table set is exp_and_others:
    zero table switches between gate activations and the attention exps.
  - every rsqrt (q/k l2-norm, rmsnorm) is computed on DVE only via the
    bit-trick + Newton iterations; the rmsnorm scale is carried as a
    per-partition column and applied at the final output evacuation.
  - scores matmuls run as row-group pairs (tile_position packing, K=64x2);
    hidden-state transposes are PE identity matmuls evacuated by DVE.
Compute dtype is bf16 (PE runs 4x slower on f32), accumulation f32 in PSUM.
"""

import os
import sys

import numpy as np

try:
    import concourse.bass as bass
except ImportError:  # staged container path
    sys.path.insert(0, "/opt/trn_rl_repo")
    import concourse.bass as bass

import ml_dtypes
from contextlib import ExitStack

import concourse.bacc as bacc
import concourse.tile as tile
from concourse import mybir
from concourse.bass_utils import run_bass_kernel_spmd
from concourse.masks import make_identity

BF16 = ml_dtypes.bfloat16
F32 = mybir.dt.float32
BF = mybir.dt.bfloat16
AF = mybir.ActivationFunctionType
ALU = mybir.AluOpType

B, L, E, D = 4, 2048, 512, 64
NCORES = 8
R = L // 2          # 1024 query rows per core
RT = R // 128       # 8 query m-tiles per core
KT = L // 128       # 16 kv m-tiles per core
EC = E // 128       # 4 contraction chunks
EPS_RMS = 1e-6
EPS_L2 = 1e-24

LAST = None  # BassKernelResults of the most recent run (for test harness)


def _build(has_bias):
    """Build the per-core SPMD program. has_bias: dict of bool flags."""
    nc = bacc.Bacc(
        "TRN2",
        target_bir_lowering=False,
        debug=False,
        enable_asserts=False,
        num_devices=NCORES,
    )

    hq_d = nc.dram_tensor("hq", [R, E], F32, kind="ExternalInput")
    hk_d = nc.dram_tensor("hk", [L, E], F32, kind="ExternalInput")
    hv_d = nc.dram_tensor("hv", [L, E], F32, kind="ExternalInput")
    hs_d = nc.dram_tensor("hs", [R, E], F32, kind="ExternalInput")
    wq_d = nc.dram_tensor("wq", [E, D], BF, kind="ExternalInput")
    wk_d = nc.dram_tensor("wk", [E, D], BF, kind="ExternalInput")
    wvb_d = nc.dram_tensor("wvb", [E, 2 * D], BF, kind="ExternalInput")
    wa1_d = nc.dram_tensor("wa1", [E, 32], BF, kind="ExternalInput")
    ws1_d = nc.dram_tensor("ws1", [E, 32], BF, kind="ExternalInput")
    wa2_d = nc.dram_tensor("wa2", [32, D], BF, kind="ExternalInput")
    ws2_d = nc.dram_tensor("ws2", [32, D], BF, kind="ExternalInput")
    wo_d = nc.dram_tensor("wo", [D, D], BF, kind="ExternalInput")
    bias_d = {}
    for name, n in [("bq", D), ("bk", D), ("bvb", 2 * D), ("ba1", 32),
                    ("ba2", D), ("bs1", 32), ("bs2", D), ("bo", D)]:
        if has_bias[name]:
            bias_d[name] = nc.dram_tensor(name, [1, n], BF, kind="ExternalInput")
    out_d = nc.dram_tensor("out", [R, D], F32, kind="ExternalOutput")

    with tile.TileContext(nc) as tc, ExitStack() as ctx:
        consts = ctx.enter_context(tc.tile_pool(name="consts", bufs=1))
        persist = ctx.enter_context(tc.tile_pool(name="persist", bufs=1))

        ident = consts.tile([128, 128], BF)
        make_identity(nc, ident)
        ones64 = consts.tile([64, 1], BF)
        nc.vector.memset(ones64, 1.0)
        ones1 = consts.tile([1, 1], BF)
        nc.vector.memset(ones1, 1.0)
        onec = consts.tile([128, D], BF)
        nc.vector.memset(onec, 1.0)
        eps_rms128 = consts.tile([128, 1], F32)
        nc.vector.memset(eps_rms128, EPS_RMS)
        magic_i = consts.tile([128, RT], mybir.dt.int32)
        nc.vector.memset(magic_i, 0x5F3759DF)
        any_bias = any(has_bias.values())
        if any_bias:
            ones_row = consts.tile([1, 512], BF)
            nc.vector.memset(ones_row, 1.0)

        def load_w(d, n, nm):
            t = consts.tile([128, EC, n], BF, name=nm)
            nc.sync.dma_start(out=t, in_=d.ap().rearrange("(c p) n -> p c n", p=128))
            return t

        wq = load_w(wq_d, D, "wq_sb")
        wk = load_w(wk_d, D, "wk_sb")
        wvb = load_w(wvb_d, 2 * D, "wvb_sb")
        wa1 = load_w(wa1_d, 32, "wa1_sb")
        ws1 = load_w(ws1_d, 32, "ws1_sb")
        wa2 = consts.tile([32, D], BF)
        nc.sync.dma_start(out=wa2, in_=wa2_d.ap())
        ws2 = consts.tile([32, D], BF)
        nc.sync.dma_start(out=ws2, in_=ws2_d.ap())
        wo = consts.tile([64, D], BF)
        nc.sync.dma_start(out=wo, in_=wo_d.ap())
        bias_sb = {}
        for name, t in bias_d.items():
            n = t.shape[1]
            bt = consts.tile([1, n], BF, name=f"{name}_sb")
            nc.sync.dma_start(out=bt, in_=t.ap())
            bias_sb[name] = bt

        def bias_mm(psum, name, cols=None):
            """Add per-column bias b[1, n] to psum accumulation via K=1 matmul."""
            if name not in bias_sb:
                return False
            b = bias_sb[name]
            if cols is not None:
                b = b[:, cols[0]:cols[1]]
            nc.tensor.matmul(psum, ones_row[:, : psum.shape[0]], b.rearrange("o n -> o n"),
                             start=False, stop=True)
            return True

        def biasT_mm(psum, name):
            """Add per-row bias (transposed layouts): psum[r, m] += b[r]."""
            if name not in bias_sb:
                return False
            nc.tensor.matmul(psum, bias_sb[name], ones_row[:, : psum.free_size()],
                             start=False, stop=True)
            return True

        # persistent SBUF tensors
        q_full = persist.tile([128, RT, D], BF)
        k_full = persist.tile([128, KT, D], BF)
        ss_q = persist.tile([128, RT], F32)
        ss_k = persist.tile([128, KT], F32)
        rs_q = persist.tile([128, RT], F32)
        rs_k = persist.tile([128, KT], F32)
        ms_cols = persist.tile([128, RT], F32)
        rs_cols = persist.tile([128, RT], F32)
        qn = persist.tile([128, RT, D], BF)
        qT2 = persist.tile([128, R], BF)   # rows 0:64 = qT, 64:128 = copy
        kT2 = persist.tile([128, KT // 2, 128], BF)  # even jt rows 0:64, odd 64:128
        vb_tanh = persist.tile([128, KT, 2 * D], BF)  # tanh(v'), tanh(b')
        v_full = persist.tile([128, KT, D], BF)
        v1 = persist.tile([128, KT, D], BF)
        a1T = persist.tile([32, L], BF)
        s1T = persist.tile([32, R], BF)
        tsc = persist.tile([64, R], BF)   # tanh of halved shortcut pre-act
        eT = persist.tile([128, KT, R], BF)
        out_sb = persist.tile([128, RT, D], F32)

        evac_ct = [0]

        def evac(dst, src):
            nc.vector.tensor_copy(dst, src)

        def rsqrt_dve(dst, src, pool, iters=2):
            """dst = 1/sqrt(src) on DVE only (Quake bit-trick + Newton).
            No ACT table set needed. src: [128, n] f32, n <= RT."""
            n = src.shape[-1]
            I32 = mybir.dt.int32
            i1 = pool.tile([128, RT], I32, tag="rqi", name="rqi")[:, :n]
            nc.vector.tensor_scalar(out=i1, in0=src.bitcast(I32), scalar1=1,
                                    scalar2=None, op0=ALU.arith_shift_right)
            x0 = pool.tile([128, RT], F32, tag="rqx", name="rqx")[:, :n]
            nc.vector.tensor_tensor(out=x0.bitcast(I32), in0=magic_i[:, :n],
                                    in1=i1, op=ALU.subtract)
            h = pool.tile([128, RT], F32, tag="rqh", name="rqh")[:, :n]
            nc.vector.tensor_scalar_mul(h, src, 0.5)
            cur = x0
            for it in range(iters):
                t = pool.tile([128, RT], F32, tag="rqt", name="rqt")[:, :n]
                nc.vector.tensor_mul(t, cur, cur)
                nc.vector.tensor_mul(t, t, h)
                nc.vector.tensor_scalar(out=t, in0=t, scalar1=-1.0,
                                        scalar2=None, op0=ALU.mult)
                nc.vector.tensor_scalar(out=t, in0=t, scalar1=1.5,
                                        scalar2=None, op0=ALU.add)
                dst_it = dst if it == iters - 1 else pool.tile(
                    [128, RT], F32, tag="rqn", name="rqn")[:, :n]
                nc.vector.tensor_mul(dst_it, cur, t)
                cur = dst_it


        # ------- processing helpers (pools passed in per scope) -------
        def load_and_transpose(src_d, blk, loadp, xtp, ps_tp):
            src = src_d.ap().rearrange("(b t p) e -> b p t e", p=128, t=4)
            nat = loadp.tile([128, 4, E], BF, tag="nat", name="nat")
            nc.gpsimd.dma_start(out=nat[:, 0:2, :], in_=src[blk, :, 0:2, :])
            nc.gpsimd.dma_start(out=nat[:, 2:4, :], in_=src[blk, :, 2:4, :])
            xt = xtp.tile([128, EC, 512], BF, tag="xt", name="xt")
            for cc in range(EC // 2):
                # two e-chunks of transposes share one psum bank (bf16)
                ps = ps_tp.tile([128, 2, 512], BF, tag="tp", name="ps_t")
                for ci in range(2):
                    c = 2 * cc + ci
                    for t in range(4):
                        nc.tensor.transpose(
                            ps[:, ci, t * 128:(t + 1) * 128],
                            nat[:, t, c * 128:(c + 1) * 128],
                            ident,
                        )
                evac_ct[0] += 1
                if evac_ct[0] % 3 == 0:
                    nc.scalar.copy(xt[:, 2 * cc:2 * cc + 2, :], ps)
                else:
                    nc.vector.tensor_copy(xt[:, 2 * cc:2 * cc + 2, :], ps)
            return xt

        def process_qk(src_d, blk, kind, loadp, xtp, sigp, ps_tp, ps_proj):
            w = wq if kind == "q" else wk
            bn = "bq" if kind == "q" else "bk"
            full = q_full if kind == "q" else k_full
            ss = ss_q if kind == "q" else ss_k
            if True:
                xt = load_and_transpose(src_d, blk, loadp, xtp, ps_tp)
                g = blk * 4
                pqk = ps_proj.tile([128, 4, D], F32, tag="proj", name="pqk")
                for h in range(4):
                    for c in range(EC):
                        nc.tensor.matmul(
                            pqk[:, h, :], xt[:, c, h * 128:(h + 1) * 128],
                            w[:, c, :], start=(c == 0),
                            stop=(c == EC - 1 and not has_bias[bn]))
                    if has_bias[bn]:
                        bias_mm(pqk[:, h, :], bn)
                sig = sigp.tile([128, 4, D], BF, tag="sig", name="sigqk")
                nc.scalar.activation(sig, pqk, AF.Tanh)
                # silu(2x') = x'*(tanh(x')+1), x' = halved pre-act
                nc.vector.scalar_tensor_tensor(
                    out=full[:, g:g + 4, :], in0=sig, scalar=1.0,
                    in1=pqk, op0=ALU.add, op1=ALU.mult)
                scr = sigp.tile([128, 4, D], F32, tag="scr", name="scr")
                nc.scalar.square(scr, full[:, g:g + 4, :])
                nc.vector.reduce_sum(
                    ss[:, g:g + 4].rearrange("p (a b) -> p a b", b=1),
                    scr, axis=mybir.AxisListType.X)
                if kind == "k":
                    # per-block rs_k so each jt-pair's exp can start early
                    rsqrt_dve(rs_k[:, g:g + 4], ss[:, g:g + 4], sigp, iters=1)

        def process_v(src_d, blk, loadp, xtp, sigp, ps_tp, ps_proj):
            if True:
                xt = load_and_transpose(src_d, blk, loadp, xtp, ps_tp)
                for u in range(2):
                    jt = blk * 4 + 2 * u
                    # two m-tiles' [v|beta] groups in one psum bank
                    pvb = ps_proj.tile([128, 2, 2 * D], F32, tag="proj", name="pvb")
                    for h in range(2):
                        for c in range(EC):
                            nc.tensor.matmul(
                                pvb[:, h, :],
                                xt[:, c, (2 * u + h) * 128:(2 * u + h + 1) * 128],
                                wvb[:, c, :], start=(c == 0),
                                stop=(c == EC - 1 and not has_bias["bvb"]))
                        bias_mm(pvb[:, h, :], "bvb")
                    nc.scalar.activation(vb_tanh[:, jt:jt + 2, :], pvb, AF.Tanh)
                    # v = x'*(tanh(x')+1) with x' = (hv@Wv + bv)/2
                    nc.vector.scalar_tensor_tensor(
                        out=v_full[:, jt:jt + 2, :],
                        in0=vb_tanh[:, jt:jt + 2, :D], scalar=1.0,
                        in1=pvb[:, :, :D], op0=ALU.add, op1=ALU.mult)
                # a1T for this block
                pa1 = ps_proj.tile([32, 512], F32, tag="proj", name="pa1")
                for c in range(EC):
                    nc.tensor.matmul(pa1, wa1[:, c, :], xt[:, c, :],
                                     start=(c == 0),
                                     stop=(c == EC - 1 and not has_bias["ba1"]))
                biasT_mm(pa1, "ba1")
                evac(a1T[:, blk * 512:(blk + 1) * 512], pa1)
                # alpha for this block, then v1' = v*(ta+1) + (tb+1) = 2*v1
                pa2 = ps_proj.tile([128, 4, D], F32, tag="proj", name="pa2")
                for h in range(4):
                    jt = blk * 4 + h
                    nc.tensor.matmul(pa2[:, h, :],
                                     a1T[:, jt * 128:(jt + 1) * 128], wa2,
                                     start=True, stop=not has_bias["ba2"])
                    if has_bias["ba2"]:
                        bias_mm(pa2[:, h, :], "ba2")
                alf = sigp.tile([128, 4, D], BF, tag="sig", name="alf")
                nc.scalar.activation(alf, pa2, AF.Tanh)
                for h in range(4):
                    jt = blk * 4 + h
                    t1 = sigp.tile([128, D], BF, tag="t1", name="t1")
                    nc.vector.scalar_tensor_tensor(
                        out=t1, in0=alf[:, h, :], scalar=1.0,
                        in1=v_full[:, jt, :], op0=ALU.add, op1=ALU.mult)
                    c1 = sigp.tile([128, D], BF, tag="c1", name="c1")
                    nc.gpsimd.tensor_add(c1, vb_tanh[:, jt, D:], onec)
                    nc.gpsimd.tensor_add(v1[:, jt, :], t1, c1)

        def process_s(src_d, blk, loadp, xtp, ps_tp, ps_proj):
            if True:
                xt = load_and_transpose(src_d, blk, loadp, xtp, ps_tp)
                ps1 = ps_proj.tile([32, 512], F32, tag="proj", name="ps1")
                for c in range(EC):
                    nc.tensor.matmul(ps1, ws1[:, c, :], xt[:, c, :],
                                     start=(c == 0),
                                     stop=(c == EC - 1 and not has_bias["bs1"]))
                biasT_mm(ps1, "bs1")
                evac(s1T[:, blk * 512:(blk + 1) * 512], ps1)

        # ---- Unified scope: per-tensor load pools + one PSUM budget so the
        # scheduler can interleave everything; hk/hv blocks alternate so both
        # the scores chain (k,q) and the v1 chain (v,alpha) finish early.
        with tc.tile_pool(name="loadk", bufs=2) as loadk, \
             tc.tile_pool(name="loadq", bufs=2) as loadq, \
             tc.tile_pool(name="loadv", bufs=2) as loadv, \
             tc.tile_pool(name="loads", bufs=2) as loads_, \
             tc.tile_pool(name="xtp", bufs=3) as xtp, \
             tc.tile_pool(name="sigp", bufs=4) as sigp, \
             tc.tile_pool(name="ps_tp", bufs=2, space="PSUM") as ps_tp, \
             tc.tile_pool(name="ps_pm", bufs=2, space="PSUM") as ps_pm, \
             tc.tile_pool(name="ps_e", bufs=2, space="PSUM") as ps_e, \
             tc.tile_pool(name="ps_sm", bufs=2, space="PSUM") as ps_sm:

            for blk in range(2):
                process_qk(hq_d, blk, "q", loadq, xtp, sigp, ps_tp, ps_pm)
            # q normalization (folded 1/8 score scale) + early qT2 transposes
            ssq64 = sigp.tile([128, RT], F32, tag="lnq", name="ssq64")
            nc.vector.tensor_scalar_mul(ssq64, ss_q, 64.0)
            rsqrt_dve(rs_q, ssq64, sigp)
            for t in range(RT):
                nc.vector.tensor_scalar_mul(qn[:, t, :], q_full[:, t, :],
                                            rs_q[:, t:t + 1])
            for t in range(RT):
                pt = ps_pm.tile([128, 128], BF, tag="proj", name="ptq")
                nc.tensor.transpose(pt[0:64, :], qn[:, t, :], ident)
                nc.tensor.transpose(pt[64:128, :], qn[:, t, :], ident,
                                    tile_position=(0, 64))
                evac(qT2[:, t * 128:(t + 1) * 128], pt)
            # k/v interleaved; kT2 transposes use RAW k (rs_k is applied later
            # as the Exp per-partition scale), so they run inline per block.
            for blk in range(4):
                process_qk(hk_d, blk, "k", loadk, xtp, sigp, ps_tp, ps_pm)
                for uu in range(2):
                    u = blk * 2 + uu
                    pt = ps_pm.tile([128, 128], BF, tag="proj", name="ptk")
                    nc.tensor.transpose(pt[0:64, :], k_full[:, 2 * u, :], ident)
                    nc.tensor.transpose(pt[64:128, :], k_full[:, 2 * u + 1, :],
                                        ident, tile_position=(0, 64))
                    evac(kT2[:, u, :], pt)
                process_v(hv_d, blk, loadv, xtp, sigp, ps_tp, ps_pm)
            # scores + exp (Tanh/Exp share the ACT table set - no thrashing)
            for u in range(KT // 2):
                for i5 in range(R // 512):
                    peA = ps_e.tile([128, 512], F32, tag="e", name="peA")
                    peB = ps_e.tile([128, 512], F32, tag="e", name="peB")
                    nc.tensor.matmul(peA, kT2[0:64, u, :],
                                     qT2[0:64, i5 * 512:(i5 + 1) * 512],
                                     start=True, stop=True,
                                     tile_position=(0, 0))
                    nc.tensor.matmul(peB, kT2[64:128, u, :],
                                     qT2[64:128, i5 * 512:(i5 + 1) * 512],
                                     start=True, stop=True,
                                     tile_position=(64, 0))
                    nc.scalar.activation(
                        eT[:, 2 * u, i5 * 512:(i5 + 1) * 512], peA, AF.Exp,
                        scale=rs_k[:, 2 * u:2 * u + 1])
                    nc.scalar.activation(
                        eT[:, 2 * u + 1, i5 * 512:(i5 + 1) * 512], peB,
                        AF.Exp, scale=rs_k[:, 2 * u + 1:2 * u + 2])

            for blk in range(2):
                process_s(hs_d, blk, loads_, xtp, ps_tp, ps_pm)
            # shortcut (transposed): tsc = tanh(halved shortcut pre-act)
            for i5 in range(R // 512):
                ps2 = ps_pm.tile([64, 512], F32, tag="proj", name="ps2")
                nc.tensor.matmul(ps2, ws2, s1T[:, i5 * 512:(i5 + 1) * 512],
                                 start=True, stop=not has_bias["bs2"])
                biasT_mm(ps2, "bs2")
                nc.scalar.activation(tsc[:, i5 * 512:(i5 + 1) * 512], ps2,
                                     AF.Tanh)

            # ---- attention + epilogue per i-block ----
            for ib in range(R // 512):
                pa = ps_sm.tile([64, 512], F32, tag="sp", name="pa")
                for jt in range(KT):
                    nc.tensor.matmul(pa, v1[:, jt, :],
                                     eT[:, jt, ib * 512:(ib + 1) * 512],
                                     start=(jt == 0), stop=(jt == KT - 1))
                # sum over d of attn_un^2, landed as per-partition columns
                # so the rmsnorm scale applies at the final evacuation.
                sq = sigp.tile([64, 512], BF, tag="sq", name="sq")
                nc.scalar.activation(sq, pa, AF.Square)
                pr = ps_sm.tile([1, 512], F32, tag="sp", name="pr")
                nc.tensor.matmul(pr, ones64, sq, start=True, stop=True)
                ssr = sigp.tile([1, 512], BF, tag="ssr", name="ssr")
                nc.vector.tensor_copy(ssr, pr)
                psc = ps_sm.tile([128, 4, 2], BF, tag="sp", name="psc")
                for tt in range(4):
                    nc.tensor.transpose(psc[:, tt, 0:1],
                                        ssr[:, tt * 128:(tt + 1) * 128],
                                        ones1)
                nc.vector.tensor_copy(ms_cols[:, ib * 4:(ib + 1) * 4],
                                      psc[:, :, 0])
                # yT' = attn_un*(tanh+1) = 2*attn_un*sc (2 folded into Wo)
                yT = sigp.tile([64, 512], BF, tag="yT", name="yT")
                nc.vector.scalar_tensor_tensor(
                    out=yT, in0=tsc[:, ib * 512:(ib + 1) * 512], scalar=1.0,
                    in1=pa, op0=ALU.add, op1=ALU.mult)
                nrm = sigp.tile([128, 4], F32, tag="nrm", name="nrm")
                nc.vector.tensor_scalar_mul(nrm, ms_cols[:, ib * 4:(ib + 1) * 4],
                                            1.0 / D)
                nc.vector.tensor_scalar(out=nrm, in0=nrm, scalar1=EPS_RMS,
                                        scalar2=None, op0=ALU.add)
                rsqrt_dve(rs_cols[:, ib * 4:(ib + 1) * 4], nrm, sigp)
                for tt in range(4):
                    g = ib * 4 + tt
                    po = ps_sm.tile([128, D], F32, tag="sp", name="po")
                    nc.tensor.matmul(po, yT[:, tt * 128:(tt + 1) * 128],
                                     wo, start=True, stop=not has_bias["bo"])
                    bias_mm(po, "bo")
                    nc.vector.tensor_scalar_mul(out_sb[:, g, :], po,
                                                rs_cols[:, g:g + 1])
                nc.sync.dma_start(
                    out=out_d.ap().rearrange("(t p) n -> p t n", p=128)[
                        :, ib * 4:(ib + 1) * 4, :],
                    in_=out_sb[:, ib * 4:(ib + 1) * 4, :],
                )

    nc.compile()
    return nc


_CACHED = None


def kernel(**inputs):
    global LAST, _CACHED
    inp = {k: np.asarray(v) for k, v in inputs.items()}

    bias_map = {"bq": "bq", "bk": "bk", "ba1": "ba1", "ba2": "ba2",
                "bs1": "bs1", "bs2": "bs2", "bo": "bo"}
    has_bias = {k: bool(np.any(inp[v])) for k, v in bias_map.items()}
    has_bias["bvb"] = bool(np.any(inp["bv"]) or np.any(inp["bb"]))

    key = tuple(sorted(has_bias.items()))
    if _CACHED is None or _CACHED[0] != key:
        _CACHED = (key, _build(has_bias))
    nc = _CACHED[1]

    bf = lambda x: np.ascontiguousarray(x.astype(BF16))
    f32 = lambda x: np.ascontiguousarray(x.astype(np.float32))
    # Gate pre-activations are halved on the host so sigmoid(x)=0.5*tanh(x/2)+0.5
    # and silu(x)=x*sigmoid(x) reduce to tanh + one scalar_tensor_tensor op.
    # The resulting global factor 2 on v1/attn cancels in rmsnorm; the factor 2
    # from the shortcut gate is folded into Wo (with g_rms).
    wo_fold = 0.5 * inp["g_rms"][:, None] * inp["Wo"]
    weights = {
        "wq": bf(0.5 * inp["Wq"]), "wk": bf(0.5 * inp["Wk"]),
        "wvb": bf(0.5 * np.concatenate([inp["Wv"], inp["Wb"]], axis=1)),
        "wa1": bf(inp["Wa1"]), "ws1": bf(inp["Ws1"]),
        "wa2": bf(0.5 * inp["Wa2"]), "ws2": bf(0.5 * inp["Ws2"]),
        "wo": bf(wo_fold),
    }
    if has_bias["bq"]:
        weights["bq"] = bf(0.5 * inp["bq"][None, :])
    if has_bias["bk"]:
        weights["bk"] = bf(0.5 * inp["bk"][None, :])
    if has_bias["bvb"]:
        weights["bvb"] = bf(0.5 * np.concatenate([inp["bv"], inp["bb"]])[None, :])
    if has_bias["ba1"]:
        weights["ba1"] = bf(inp["ba1"][None, :])
    if has_bias["ba2"]:
        weights["ba2"] = bf(0.5 * inp["ba2"][None, :])
    if has_bias["bs1"]:
        weights["bs1"] = bf(inp["bs1"][None, :])
    if has_bias["bs2"]:
        weights["bs2"] = bf(0.5 * inp["bs2"][None, :])
    if has_bias["bo"]:
        weights["bo"] = bf(inp["bo"][None, :])

    in_maps = []
    for c in range(NCORES):
        b, h = c // 2, c % 2
        m = dict(weights)
        m["hq"] = f32(inp["hidden_query"][b, h * R:(h + 1) * R])
        m["hk"] = f32(inp["hidden_key"][b])
        m["hv"] = f32(inp["hidden_value"][b])
        m["hs"] = f32(inp["hidden_shortcut"][b, h * R:(h + 1) * R])
        in_maps.append(m)

    LAST = run_bass_kernel_spmd(nc, in_maps, core_ids=list(range(NCORES)))

    out = np.empty((B, L, D), np.float32)
    for c in range(NCORES):
        b, h = c // 2, c % 2
        out[b, h * R:(h + 1) * R] = LAST.results[c]["out"]
    return out


if __name__ == "__main__":
    rng = np.random.default_rng(0)
    fake = {}
    fake["hidden_query"] = rng.standard_normal((B, L, E), dtype=np.float32)
    fake["hidden_key"] = rng.standard_normal((B, L, E), dtype=np.float32)
    fake["hidden_value"] = rng.standard_normal((B, L, E), dtype=np.float32)
    fake["hidden_shortcut"] = rng.standard_normal((B, L, E), dtype=np.float32)
    for n, s in [("Wq", (E, D)), ("Wk", (E, D)), ("Wv", (E, D)), ("Wa1", (E, 32)),
                 ("Wa2", (32, D)), ("Wb", (E, D)), ("Ws1", (E, 32)), ("Ws2", (32, D)),
                 ("Wo", (D, D))]:
        fake[n] = rng.standard_normal(s, dtype=np.float32) * 0.05
    for n, s in [("bq", D), ("bk", D), ("bv", D), ("ba1", 32), ("ba2", D),
                 ("bb", D), ("bs1", 32), ("bs2", D), ("bo", D)]:
        fake[n] = np.zeros(s, np.float32)
    fake["g_rms"] = np.ones(D, np.float32)
    o = kernel(**fake)
    print("ran:", o.shape, o.dtype, np.abs(o).max())



# revision 8
# speedup vs baseline: 1.2936x; 1.2936x over previous
"""Trainium2 Bass kernel for gated single-head attention (B=4, L=2048, E=512, D=64).

Sharding: data-parallel over 8 cores; core c handles batch b=c//2, query-row
half h=c%2 (1024 query rows). hk/hv are processed per-core for the full batch.

Math restructuring (validated in numpy against the jax reference):
  - q,k are L2-normalized so scores s = (q^.k^)/8 lie in [-1/8, 1/8]; softmax
    exp is linearized: e = 1 + s (rel err 6e-6 after rmsnorm, which cancels
    the near-uniform quadratic term). The attention then COLLAPSES to a
    64x64 bilinear form:
        attn[i] = P0 + rs_q_i * (G^T q_i),
        G = sum_j (k_j/|k_j|) (x) v1_j   [64x64],  P0 = sum_j v1_j.
    No 2048x1024 score matrix, no exp, no per-score evacuation.
  - rs_q (1/(8|q_i|)) and the rmsnorm scale are per-query; both are applied
    AFTER the final Wo projection (queries land on partitions there), using
    rmsnorm(P0 + r*P1) algebra: ms*64 = c0 + 2*r*u + r^2*w with
    c0 = |P0|^2, u = P0.P1_i, w = |P1_i|^2 - all computed by tiny matmuls.
  - inputs are pre-transposed AND pre-cast to bf16 on the host: zero PE
    transposes for the projections, and half the HBM traffic.
  - all gates go through Tanh (sigma(x) = tanh(x/2)/2 + 1/2, the 1/2s folded
    into host-side weights); every rsqrt is DVE-only (bit-trick + Newton).
Compute dtype bf16, accumulation f32 in PSUM.
"""

import os
import sys

import numpy as np

try:
    import concourse.bass as bass
except ImportError:  # staged container path
    sys.path.insert(0, "/opt/trn_rl_repo")
    import concourse.bass as bass

import ml_dtypes
from contextlib import ExitStack

import concourse.bacc as bacc
import concourse.tile as tile
from concourse import mybir
from concourse.bass_utils import run_bass_kernel_spmd

BF16 = ml_dtypes.bfloat16
F32 = mybir.dt.float32
BF = mybir.dt.bfloat16
AF = mybir.ActivationFunctionType
ALU = mybir.AluOpType

B, L, E, D = 4, 2048, 512, 64
NCORES = 8
R = L // 2          # 1024 query rows per core
RT = R // 128       # 8 query m-tiles per core
KT = L // 128       # 16 kv m-tiles per core
EC = E // 128       # 4 contraction chunks
QC = 2              # query chunks of 512
EPS_RMS = 4e-6      # 1e-6 * 4 (v1 carries a global factor 2)

LAST = None  # BassKernelResults of the most recent run (for test harness)


def _build(has_bias):
    """Build the per-core SPMD program. has_bias: dict of bool flags."""
    nc = bacc.Bacc(
        "TRN2",
        target_bir_lowering=False,
        debug=False,
        enable_asserts=False,
        num_devices=NCORES,
    )

    hqT_d = nc.dram_tensor("hqT", [E, R], BF, kind="ExternalInput")
    hkT_d = nc.dram_tensor("hkT", [E, L], BF, kind="ExternalInput")
    hvT_d = nc.dram_tensor("hvT", [E, L], BF, kind="ExternalInput")
    hsT_d = nc.dram_tensor("hsT", [E, R], BF, kind="ExternalInput")
    wq_d = nc.dram_tensor("wq", [E, D], BF, kind="ExternalInput")
    wk_d = nc.dram_tensor("wk", [E, D], BF, kind="ExternalInput")
    wvb_d = nc.dram_tensor("wvb", [E, 2 * D], BF, kind="ExternalInput")
    wa1_d = nc.dram_tensor("wa1", [E, 32], BF, kind="ExternalInput")
    ws1_d = nc.dram_tensor("ws1", [E, 32], BF, kind="ExternalInput")
    wa2_d = nc.dram_tensor("wa2", [32, D], BF, kind="ExternalInput")
    ws2_d = nc.dram_tensor("ws2", [32, D], BF, kind="ExternalInput")
    wo_d = nc.dram_tensor("wo", [D, D], BF, kind="ExternalInput")
    bias_d = {}
    for name, n in [("bq", D), ("bk", D), ("bvb", 2 * D), ("ba1", 32),
                    ("ba2", D), ("bs1", 32), ("bs2", D), ("bo", D)]:
        if has_bias[name]:
            bias_d[name] = nc.dram_tensor(name, [1, n], BF, kind="ExternalInput")
    out_d = nc.dram_tensor("out", [R, D], F32, kind="ExternalOutput")

    with tile.TileContext(nc) as tc, ExitStack() as ctx:
        consts = ctx.enter_context(tc.tile_pool(name="consts", bufs=1))
        persist = ctx.enter_context(tc.tile_pool(name="persist", bufs=1))

        ones128c = consts.tile([128, 1], BF)
        nc.vector.memset(ones128c, 1.0)
        ones64 = consts.tile([64, 1], BF)
        nc.vector.memset(ones64, 1.0)
        ones64x128 = consts.tile([64, 128], BF)
        nc.vector.memset(ones64x128, 1.0)
        magic_i = consts.tile([128, KT], mybir.dt.int32)
        nc.vector.memset(magic_i, 0x5F3759DF)
        any_bias = any(has_bias.values())
        if any_bias:
            ones_row = consts.tile([1, 512], BF)
            nc.vector.memset(ones_row, 1.0)

        def load_w(d, n, nm):
            t = consts.tile([128, EC, n], BF, name=nm)
            nc.sync.dma_start(out=t, in_=d.ap().rearrange("(c p) n -> p c n", p=128))
            return t

        wq = load_w(wq_d, D, "wq_sb")
        wk = load_w(wk_d, D, "wk_sb")
        wvb = load_w(wvb_d, 2 * D, "wvb_sb")
        wa1 = load_w(wa1_d, 32, "wa1_sb")
        ws1 = load_w(ws1_d, 32, "ws1_sb")
        wa2 = consts.tile([32, D], BF)
        nc.sync.dma_start(out=wa2, in_=wa2_d.ap())
        ws2 = consts.tile([32, D], BF)
        nc.sync.dma_start(out=ws2, in_=ws2_d.ap())
        wo = consts.tile([64, D], BF)
        nc.sync.dma_start(out=wo, in_=wo_d.ap())
        bias_sb = {}
        for name, t in bias_d.items():
            n = t.shape[1]
            bt = consts.tile([1, n], BF, name=f"{name}_sb")
            nc.sync.dma_start(out=bt, in_=t.ap())
            bias_sb[name] = bt

        def bias_mm(psum, name):
            """Add per-column bias b[1, n] to psum accumulation via K=1 matmul."""
            if name not in bias_sb:
                return False
            nc.tensor.matmul(psum, ones_row[:, : psum.shape[0]], bias_sb[name],
                             start=False, stop=True)
            return True

        def biasT_mm(psum, name):
            """Add per-row bias (transposed layouts): psum[r, m] += b[r]."""
            if name not in bias_sb:
                return False
            nc.tensor.matmul(psum, bias_sb[name], ones_row[:, : psum.free_size()],
                             start=False, stop=True)
            return True

        # persistent SBUF tensors
        k2o = persist.tile([128, KT, D + 1], BF)    # k/|k| plus a ones column
        nc.vector.memset(k2o[:, :, D:D + 1], 1.0)
        v1 = persist.tile([128, KT, D], BF)
        ss_k = persist.tile([128, KT], F32)
        rs_k = persist.tile([128, KT], F32)
        Gfull = persist.tile([128, D], BF)          # rows 0:64 = G, row 64 = P0
        P0col = persist.tile([64, 1], BF)
        P0col_f = persist.tile([64, 1], F32)
        sqP0 = persist.tile([64, 1], BF)
        c0_c = persist.tile([128, 1], F32)
        cols = persist.tile([128, RT, 3], F32)      # ssq / u / w per query tile
        rq_c = persist.tile([128, RT], F32)
        rms_c = persist.tile([128, RT], F32)
        rmsq_c = persist.tile([128, RT], F32)
        out_sb = persist.tile([128, RT, D], F32)

        def rsqrt_dve(dst, src, pool, iters=2):
            """dst = 1/sqrt(src) on DVE only (Quake bit-trick + Newton).
            src: [128, n] f32, n <= KT."""
            n = src.shape[-1]
            I32 = mybir.dt.int32
            i1 = pool.tile([128, KT], I32, tag="rqi", name="rqi")[:, :n]
            nc.vector.tensor_scalar(out=i1, in0=src.bitcast(I32), scalar1=1,
                                    scalar2=None, op0=ALU.arith_shift_right)
            x0 = pool.tile([128, KT], F32, tag="rqx", name="rqx")[:, :n]
            nc.vector.tensor_tensor(out=x0.bitcast(I32), in0=magic_i[:, :n],
                                    in1=i1, op=ALU.subtract)
            h = pool.tile([128, KT], F32, tag="rqh", name="rqh")[:, :n]
            nc.vector.tensor_scalar_mul(h, src, 0.5)
            cur = x0
            for it in range(iters):
                t = pool.tile([128, KT], F32, tag="rqt", name="rqt")[:, :n]
                nc.vector.tensor_mul(t, cur, cur)
                nc.vector.tensor_mul(t, t, h)
                nc.vector.tensor_scalar(out=t, in0=t, scalar1=-1.0,
                                        scalar2=None, op0=ALU.mult)
                nc.vector.tensor_scalar(out=t, in0=t, scalar1=1.5,
                                        scalar2=None, op0=ALU.add)
                dst_it = dst if it == iters - 1 else pool.tile(
                    [128, KT], F32, tag="rqn", name="rqn")[:, :n]
                nc.vector.tensor_mul(dst_it, cur, t)
                cur = dst_it

        with tc.tile_pool(name="loadk", bufs=2) as loadk, \
             tc.tile_pool(name="loadv", bufs=2) as loadv, \
             tc.tile_pool(name="loadq", bufs=2) as loadq, \
             tc.tile_pool(name="loads", bufs=2) as loads_, \
             tc.tile_pool(name="sig", bufs=4) as sig, \
             tc.tile_pool(name="psA", bufs=3, space="PSUM") as psA, \
             tc.tile_pool(name="psW", bufs=2, space="PSUM") as psW, \
             tc.tile_pool(name="psG", bufs=1, space="PSUM") as psG, \
             tc.tile_pool(name="psT", bufs=2, space="PSUM") as psT:

            G_ps = psG.tile([128, D], F32, name="G_ps")

            # ================= k/v phase: 4 blocks of 512 keys =================
            hkT_src = hkT_d.ap().rearrange("(c p) r -> p c r", p=128)
            hvT_src = hvT_d.ap().rearrange("(c p) r -> p c r", p=128)
            for blk in range(4):
                ks = slice(blk * 512, (blk + 1) * 512)
                hkb = loadk.tile([128, EC, 512], BF, tag="hk", name="hkb")
                nc.gpsimd.dma_start(out=hkb, in_=hkT_src[:, :, ks])
                hvb = loadv.tile([128, EC, 512], BF, tag="hv", name="hvb")
                nc.gpsimd.dma_start(out=hvb, in_=hvT_src[:, :, ks])

                # ---- k projection (row-major) + silu + |k| ----
                pk = psA.tile([128, 4, D], F32, tag="proj", name="pk")
                for t in range(4):
                    for c in range(EC):
                        nc.tensor.matmul(
                            pk[:, t, :], hkb[:, c, t * 128:(t + 1) * 128],
                            wk[:, c, :], start=(c == 0),
                            stop=(c == EC - 1 and not has_bias["bk"]))
                    bias_mm(pk[:, t, :], "bk")
                ktan = sig.tile([128, 4, D], BF, tag="sig", name="ktan")
                nc.scalar.activation(ktan, pk, AF.Tanh)
                kf = sig.tile([128, 4, D], BF, tag="kf", name="kf")
                nc.vector.scalar_tensor_tensor(
                    out=kf, in0=ktan, scalar=1.0, in1=pk,
                    op0=ALU.add, op1=ALU.mult)
                ksq = sig.tile([128, 4, D], BF, tag="ksq", name="ksq")
                g = blk * 4
                for t in range(4):
                    nc.scalar.activation(ksq[:, t, :], kf[:, t, :], AF.Square,
                                         accum_out=ss_k[:, g + t:g + t + 1])
                rsqrt_dve(rs_k[:, g:g + 4], ss_k[:, g:g + 4], sig, iters=2)
                for t in range(4):
                    nc.vector.tensor_scalar_mul(
                        k2o[:, g + t, :D], kf[:, t, :], rs_k[:, g + t:g + t + 1])

                # ---- v | beta projection + silu ----
                vbt = sig.tile([128, 4, 2 * D], BF, tag="vbt", name="vbt")
                vf = sig.tile([128, 4, D], BF, tag="vf", name="vf")
                for u in range(2):
                    pvb = psA.tile([128, 2, 2 * D], F32, tag="proj", name="pvb")
                    for hh in range(2):
                        t = 2 * u + hh
                        for c in range(EC):
                            nc.tensor.matmul(
                                pvb[:, hh, :],
                                hvb[:, c, t * 128:(t + 1) * 128],
                                wvb[:, c, :], start=(c == 0),
                                stop=(c == EC - 1 and not has_bias["bvb"]))
                        bias_mm(pvb[:, hh, :], "bvb")
                    nc.scalar.activation(vbt[:, 2 * u:2 * u + 2, :], pvb, AF.Tanh)
                    nc.vector.scalar_tensor_tensor(
                        out=vf[:, 2 * u:2 * u + 2, :],
                        in0=vbt[:, 2 * u:2 * u + 2, :D], scalar=1.0,
                        in1=pvb[:, :, :D], op0=ALU.add, op1=ALU.mult)

                # ---- alpha: a1T (weight-stationary) then a2 (row-major) ----
                pa1 = psA.tile([32, 512], F32, tag="proj", name="pa1")
                for c in range(EC):
                    nc.tensor.matmul(pa1, wa1[:, c, :], hvb[:, c, :],
                                     start=(c == 0),
                                     stop=(c == EC - 1 and not has_bias["ba1"]))
                biasT_mm(pa1, "ba1")
                a1T = sig.tile([32, 512], BF, tag="a1T", name="a1T")
                nc.vector.tensor_copy(a1T, pa1)
                pa2 = psA.tile([128, 4, D], F32, tag="proj", name="pa2")
                for t in range(4):
                    nc.tensor.matmul(pa2[:, t, :],
                                     a1T[:, t * 128:(t + 1) * 128], wa2,
                                     start=True, stop=not has_bias["ba2"])
                    bias_mm(pa2[:, t, :], "ba2")
                alf = sig.tile([128, 4, D], BF, tag="sig", name="alf")
                nc.scalar.activation(alf, pa2, AF.Tanh)
                # v1 = vf*(alf+1) + (vbt_beta+1)   (= 2*(v*alpha+beta))
                t1 = sig.tile([128, 4, D], BF, tag="t1", name="t1")
                nc.vector.scalar_tensor_tensor(
                    out=t1, in0=alf, scalar=1.0, in1=vf,
                    op0=ALU.add, op1=ALU.mult)
                nc.vector.scalar_tensor_tensor(
                    out=v1[:, g:g + 4, :], in0=vbt[:, :, D:], scalar=1.0,
                    in1=t1, op0=ALU.add, op1=ALU.add)

                # ---- G accumulation: G[0:64] += k2^T v1, G[64] += sum v1 ----
                for t in range(4):
                    jt = g + t
                    nc.tensor.matmul(G_ps[0:65, :], k2o[:, jt, :],
                                     v1[:, jt, :], start=(jt == 0),
                                     stop=(jt == KT - 1))

            # ---- G epilogue: evacuate, P0 column, c0 ----
            nc.vector.tensor_copy(Gfull[0:65, :], G_ps[0:65, :])
            p0c_ps = psT.tile([64, 1], BF, tag="tp", name="p0c")
            nc.tensor.transpose(p0c_ps, Gfull[64:65, 0:D], ones128c[64:65, :])
            nc.vector.tensor_copy(P0col, p0c_ps)
            nc.vector.tensor_copy(P0col_f, p0c_ps)
            nc.scalar.activation(sqP0, P0col, AF.Square)
            c0_ps = psT.tile([128, 1], F32, tag="tp", name="c0ps")
            nc.tensor.matmul(c0_ps, ones64x128, sqP0, start=True, stop=True)
            nc.vector.tensor_copy(c0_c, c0_ps)

            # ================= query phase: 2 chunks of 512 =================
            hqT_src = hqT_d.ap().rearrange("(c p) r -> p c r", p=128)
            hsT_src = hsT_d.ap().rearrange("(c p) r -> p c r", p=128)
            for qc in range(QC):
                qs = slice(qc * 512, (qc + 1) * 512)
                hsb = loads_.tile([128, EC, 512], BF, tag="hs", name="hsb")
                nc.sync.dma_start(out=hsb, in_=hsT_src[:, :, qs])
                hqb = loadq.tile([128, EC, 512], BF, tag="hq", name="hqb")
                nc.sync.dma_start(out=hqb, in_=hqT_src[:, :, qs])

                # ---- shortcut gate tanh (transposed): tsc [64, 512] ----
                ps1 = psA.tile([32, 512], F32, tag="proj", name="ps1")
                for c in range(EC):
                    nc.tensor.matmul(ps1, ws1[:, c, :], hsb[:, c, :],
                                     start=(c == 0),
                                     stop=(c == EC - 1 and not has_bias["bs1"]))
                biasT_mm(ps1, "bs1")
                s1T = sig.tile([32, 512], BF, tag="a1T", name="s1T")
                nc.vector.tensor_copy(s1T, ps1)
                ps2 = psA.tile([64, 512], F32, tag="proj", name="ps2")
                nc.tensor.matmul(ps2, ws2, s1T,
                                 start=True, stop=not has_bias["bs2"])
                biasT_mm(ps2, "bs2")
                tsc = sig.tile([64, 512], BF, tag="tsc", name="tsc")
                nc.scalar.activation(tsc, ps2, AF.Tanh)

                # ---- qT projection (transposed) + silu ----
                pq = psW.tile([64, 512], F32, tag="pw", name="pq")
                for c in range(EC):
                    nc.tensor.matmul(pq, wq[:, c, :], hqb[:, c, :],
                                     start=(c == 0),
                                     stop=(c == EC - 1 and not has_bias["bq"]))
                biasT_mm(pq, "bq")
                qtan = sig.tile([64, 512], BF, tag="qtan", name="qtan")
                nc.scalar.activation(qtan, pq, AF.Tanh)
                qT = sig.tile([64, 512], BF, tag="qT", name="qT")
                nc.vector.scalar_tensor_tensor(
                    out=qT, in0=qtan, scalar=1.0, in1=pq,
                    op0=ALU.add, op1=ALU.mult)
                sqq = sig.tile([64, 512], BF, tag="sqq", name="sqq")
                nc.scalar.activation(sqq, qT, AF.Square)

                # ---- P1 = G^T qT ----
                pP1 = psW.tile([64, 512], F32, tag="pw", name="pP1")
                nc.tensor.matmul(pP1, Gfull[0:64, :], qT, start=True, stop=True)
                P1sb = sig.tile([64, 512], BF, tag="P1sb", name="P1sb")
                nc.vector.tensor_copy(P1sb, pP1)
                sqP1 = sig.tile([64, 512], BF, tag="sqP1", name="sqP1")
                nc.scalar.activation(sqP1, pP1, AF.Square)

                # ---- per-query-tile reductions: ssq / u / w columns ----
                for tt in range(4):
                    idx = qc * 4 + tt
                    cs = slice(tt * 128, (tt + 1) * 128)
                    pcols = psT.tile([128, 3], F32, tag="tp", name="pcols")
                    nc.tensor.matmul(pcols[:, 0:1], sqq[:, cs], ones64,
                                     start=True, stop=True)
                    nc.tensor.matmul(pcols[:, 1:2], P1sb[:, cs], P0col,
                                     start=True, stop=True)
                    nc.tensor.matmul(pcols[:, 2:3], sqP1[:, cs], ones64,
                                     start=True, stop=True)
                    nc.vector.tensor_copy(cols[:, idx, :], pcols)

                # ---- rmsnorm algebra on [128, 4] columns ----
                csl = slice(qc * 4, (qc + 1) * 4)
                ssq64 = sig.tile([128, 4], F32, tag="ep1", name="ssq64")
                nc.vector.tensor_scalar_mul(ssq64, cols[:, csl, 0], 64.0)
                rsqrt_dve(rq_c[:, csl], ssq64, sig, iters=2)
                tA = sig.tile([128, 4], F32, tag="ep2", name="tA")
                nc.vector.tensor_mul(tA, rq_c[:, csl], cols[:, csl, 1])
                tB = sig.tile([128, 4], F32, tag="ep3", name="tB")
                nc.vector.tensor_mul(tB, rq_c[:, csl], rq_c[:, csl])
                nc.vector.tensor_mul(tB, tB, cols[:, csl, 2])
                # ms64 = c0 + 2*tA + tB
                nc.vector.scalar_tensor_tensor(
                    out=tB, in0=tA, scalar=2.0, in1=tB,
                    op0=ALU.mult, op1=ALU.add)
                nc.vector.tensor_scalar(out=tB, in0=tB, scalar1=c0_c,
                                        scalar2=None, op0=ALU.add)
                nc.vector.tensor_scalar(out=tB, in0=tB, scalar1=1.0 / 64,
                                        scalar2=EPS_RMS, op0=ALU.mult,
                                        op1=ALU.add)
                rsqrt_dve(rms_c[:, csl], tB, sig, iters=2)
                nc.vector.tensor_mul(rmsq_c[:, csl], rms_c[:, csl], rq_c[:, csl])

                # ---- yTA/yTB and the final Wo projections ----
                scp = sig.tile([64, 512], BF, tag="scp", name="scp")
                nc.vector.tensor_scalar(out=scp, in0=tsc, scalar1=1.0,
                                        scalar2=None, op0=ALU.add)
                yTA = sig.tile([64, 512], BF, tag="yTA", name="yTA")
                nc.vector.tensor_scalar_mul(yTA, scp, P0col_f)
                yTB = sig.tile([64, 512], BF, tag="yTB", name="yTB")
                nc.vector.tensor_mul(yTB, scp, P1sb)
                for tt in range(4):
                    idx = qc * 4 + tt
                    cs = slice(tt * 128, (tt + 1) * 128)
                    po2 = psT.tile([128, 2, D], F32, tag="tp", name="po2")
                    nc.tensor.matmul(po2[:, 0, :], yTA[:, cs], wo,
                                     start=True, stop=not has_bias["bo"])
                    bias_mm(po2[:, 0, :], "bo")
                    nc.tensor.matmul(po2[:, 1, :], yTB[:, cs], wo,
                                     start=True, stop=True)
                    tmp = sig.tile([128, D], F32, tag="tmp", name="tmp")
                    nc.vector.tensor_scalar_mul(tmp, po2[:, 0, :],
                                                rms_c[:, idx:idx + 1])
                    nc.vector.scalar_tensor_tensor(
                        out=out_sb[:, idx, :], in0=po2[:, 1, :],
                        scalar=rmsq_c[:, idx:idx + 1], in1=tmp,
                        op0=ALU.mult, op1=ALU.add)
                nc.sync.dma_start(
                    out=out_d.ap().rearrange("(t p) n -> p t n", p=128)[
                        :, csl, :],
                    in_=out_sb[:, csl, :],
                )

    nc.compile()
    return nc


_CACHED = None


def kernel(**inputs):
    global LAST, _CACHED
    inp = {k: np.asarray(v) for k, v in inputs.items()}

    bias_map = {"bq": "bq", "bk": "bk", "ba1": "ba1", "ba2": "ba2",
                "bs1": "bs1", "bs2": "bs2", "bo": "bo"}
    has_bias = {k: bool(np.any(inp[v])) for k, v in bias_map.items()}
    has_bias["bvb"] = bool(np.any(inp["bv"]) or np.any(inp["bb"]))

    key = tuple(sorted(has_bias.items()))
    if _CACHED is None or _CACHED[0] != key:
        _CACHED = (key, _build(has_bias))
    nc = _CACHED[1]

    bf = lambda x: np.ascontiguousarray(x.astype(BF16))
    bfT = lambda x: np.ascontiguousarray(x.astype(BF16).T)
    # Gate pre-activations are halved on the host so sigmoid(x)=0.5*tanh(x/2)+0.5
    # and silu(x)=x*sigmoid(x) reduce to tanh + one scalar_tensor_tensor op.
    # The resulting global factor 2 on v1/attn cancels in rmsnorm; the factor 2
    # from the shortcut gate is folded into Wo (with g_rms).
    wo_fold = 0.5 * inp["g_rms"][:, None] * inp["Wo"]
    weights = {
        "wq": bf(0.5 * inp["Wq"]), "wk": bf(0.5 * inp["Wk"]),
        "wvb": bf(0.5 * np.concatenate([inp["Wv"], inp["Wb"]], axis=1)),
        "wa1": bf(inp["Wa1"]), "ws1": bf(inp["Ws1"]),
        "wa2": bf(0.5 * inp["Wa2"]), "ws2": bf(0.5 * inp["Ws2"]),
        "wo": bf(wo_fold),
    }
    if has_bias["bq"]:
        weights["bq"] = bf(0.5 * inp["bq"][None, :])
    if has_bias["bk"]:
        weights["bk"] = bf(0.5 * inp["bk"][None, :])
    if has_bias["bvb"]:
        weights["bvb"] = bf(0.5 * np.concatenate([inp["bv"], inp["bb"]])[None, :])
    if has_bias["ba1"]:
        weights["ba1"] = bf(inp["ba1"][None, :])
    if has_bias["ba2"]:
        weights["ba2"] = bf(0.5 * inp["ba2"][None, :])
    if has_bias["bs1"]:
        weights["bs1"] = bf(inp["bs1"][None, :])
    if has_bias["bs2"]:
        weights["bs2"] = bf(0.5 * inp["bs2"][None, :])
    if has_bias["bo"]:
        weights["bo"] = bf(inp["bo"][None, :])

    in_maps = []
    for c in range(NCORES):
        b, h = c // 2, c % 2
        m = dict(weights)
        m["hqT"] = bfT(inp["hidden_query"][b, h * R:(h + 1) * R])
        m["hkT"] = bfT(inp["hidden_key"][b])
        m["hvT"] = bfT(inp["hidden_value"][b])
        m["hsT"] = bfT(inp["hidden_shortcut"][b, h * R:(h + 1) * R])
        in_maps.append(m)

    LAST = run_bass_kernel_spmd(nc, in_maps, core_ids=list(range(NCORES)))

    out = np.empty((B, L, D), np.float32)
    for c in range(NCORES):
        b, h = c // 2, c % 2
        out[b, h * R:(h + 1) * R] = LAST.results[c]["out"]
    return out


if __name__ == "__main__":
    rng = np.random.default_rng(0)
    fake = {}
    fake["hidden_query"] = rng.standard_normal((B, L, E), dtype=np.float32)
    fake["hidden_key"] = rng.standard_normal((B, L, E), dtype=np.float32)
    fake["hidden_value"] = rng.standard_normal((B, L, E), dtype=np.float32)
    fake["hidden_shortcut"] = rng.standard_normal((B, L, E), dtype=np.float32)
    for n, s in [("Wq", (E, D)), ("Wk", (E, D)), ("Wv", (E, D)), ("Wa1", (E, 32)),
                 ("Wa2", (32, D)), ("Wb", (E, D)), ("Ws1", (E, 32)), ("Ws2", (32, D)),
                 ("Wo", (D, D))]:
        fake[n] = rng.standard_normal(s, dtype=np.float32) * 0.05
    for n, s in [("bq", D), ("bk", D), ("bv", D), ("ba1", 32), ("ba2", D),
                 ("bb", D), ("bs1", 32), ("bs2", D), ("bo", D)]:
        fake[n] = np.zeros(s, np.float32)
    fake["g_rms"] = np.ones(D, np.float32)
    o = kernel(**fake)
    print("ran:", o.shape, o.dtype, np.abs(o).max())


# revision 14
# speedup vs baseline: 1.2947x; 1.0008x over previous
"""Trainium2 Bass kernel for gated single-head attention (B=4, L=2048, E=512, D=64).

Sharding: data-parallel over 8 cores; core c handles batch b=c//2, query-row
half h=c%2 (1024 query rows). hk/hv are processed per-core for the full batch.

Math restructuring (validated in numpy against the jax reference):
  - q,k are L2-normalized so scores s = (q^.k^)/8 lie in [-1/8, 1/8]; softmax
    exp is linearized: e = 1 + s (rel err 6e-6 after rmsnorm, which cancels
    the near-uniform quadratic term). The attention then COLLAPSES to a
    64x64 bilinear form:
        attn[i] = P0 + rs_q_i * (G^T q_i),
        G = sum_j (k_j/|k_j|) (x) v1_j   [64x64],  P0 = sum_j v1_j.
    No 2048x1024 score matrix, no exp, no per-score evacuation.
  - rs_q (1/(8|q_i|)) and the rmsnorm scale are per-query; both are applied
    AFTER the final Wo projection (queries land on partitions there), using
    rmsnorm(P0 + r*P1) algebra: ms*64 = c0 + 2*r*u + r^2*w with
    c0 = |P0|^2, u = P0.P1_i, w = |P1_i|^2 - all computed by tiny matmuls.
  - inputs are pre-transposed AND pre-cast to bf16 on the host: zero PE
    transposes for the projections, and half the HBM traffic.
  - the query path is stacked two 512-query halves on 128 partitions
    (G / P0 / wo duplicated into partitions 64:128, matmuls use quadrant
    tile positions) so every elementwise op runs at full 128-lane rate.
  - gates go through Tanh (sigma(x) = tanh(x/2)/2 + 1/2, the 1/2s folded
    into host-side weights); in-phase rsqrts are DVE/Pool bit-trick Newton;
    the tail switches the ACT table once (dummy Sqrt) and uses
    Sqrt + vector.reciprocal.
Compute dtype bf16, accumulation f32 in PSUM.
"""

import os
import sys

import numpy as np

try:
    import concourse.bass as bass
except ImportError:  # staged container path
    sys.path.insert(0, "/opt/trn_rl_repo")
    import concourse.bass as bass

import ml_dtypes
from contextlib import ExitStack

import concourse.bacc as bacc
import concourse.tile as tile
from concourse import mybir
from concourse.bass_utils import run_bass_kernel_spmd
from concourse.masks import make_identity

BF16 = ml_dtypes.bfloat16
F32 = mybir.dt.float32
BF = mybir.dt.bfloat16
AF = mybir.ActivationFunctionType
ALU = mybir.AluOpType

B, L, E, D = 4, 2048, 512, 64
NCORES = 8
R = L // 2          # 1024 query rows per core
RT = R // 128       # 8 query m-tiles per core
KT = L // 128       # 16 kv m-tiles per core
EC = E // 128       # 4 contraction chunks
EPS_RMS = 4e-6      # 1e-6 * 4 (v1 carries a global factor 2)

LAST = None  # BassKernelResults of the most recent run (for test harness)


def _build(has_bias):
    """Build the per-core SPMD program. has_bias: dict of bool flags."""
    nc = bacc.Bacc(
        "TRN2",
        target_bir_lowering=False,
        debug=False,
        enable_asserts=False,
        num_devices=NCORES,
    )

    hqT_d = nc.dram_tensor("hqT", [E, R], BF, kind="ExternalInput")
    hkT_d = nc.dram_tensor("hkT", [E, L], BF, kind="ExternalInput")
    hvT_d = nc.dram_tensor("hvT", [E, L], BF, kind="ExternalInput")
    hsT_d = nc.dram_tensor("hsT", [E, R], BF, kind="ExternalInput")
    wq_d = nc.dram_tensor("wq", [E, D], BF, kind="ExternalInput")
    wk_d = nc.dram_tensor("wk", [E, D], BF, kind="ExternalInput")
    wvb_d = nc.dram_tensor("wvb", [E, 2 * D], BF, kind="ExternalInput")
    wa1_d = nc.dram_tensor("wa1", [E, 32], BF, kind="ExternalInput")
    ws1_d = nc.dram_tensor("ws1", [E, 32], BF, kind="ExternalInput")
    wa2_d = nc.dram_tensor("wa2", [32, D], BF, kind="ExternalInput")
    ws2_d = nc.dram_tensor("ws2", [32, D], BF, kind="ExternalInput")
    wo_d = nc.dram_tensor("wo", [D, D], BF, kind="ExternalInput")
    bias_d = {}
    for name, n in [("bq", D), ("bk", D), ("bvb", 2 * D), ("ba1", 32),
                    ("ba2", D), ("bs1", 32), ("bs2", D), ("bo", D)]:
        if has_bias[name]:
            bias_d[name] = nc.dram_tensor(name, [1, n], BF, kind="ExternalInput")
    out_d = nc.dram_tensor("out", [R, D], F32, kind="ExternalOutput")

    with tile.TileContext(nc) as tc, ExitStack() as ctx:
        consts = ctx.enter_context(tc.tile_pool(name="consts", bufs=1))
        persist = ctx.enter_context(tc.tile_pool(name="persist", bufs=1))

        ones128c = consts.tile([128, 1], BF)
        nc.vector.memset(ones128c, 1.0)
        ones64x128 = consts.tile([64, 128], BF)
        nc.vector.memset(ones64x128, 1.0)
        onef = consts.tile([1, 1], F32)
        nc.vector.memset(onef, 1.0)
        eps128 = consts.tile([128, 1], F32)
        nc.vector.memset(eps128, EPS_RMS)
        ident64 = consts.tile([64, 64], BF)
        make_identity(nc, ident64)
        magic_i = consts.tile([128, KT], mybir.dt.int32)
        nc.vector.memset(magic_i, 0x5F3759DF)
        any_bias = any(has_bias.values())
        if any_bias:
            ones_row = consts.tile([1, 512], BF)
            nc.vector.memset(ones_row, 1.0)

        # --- weights: kv-path weights early on sync; the rest on scalar ---
        def load_w(d, n, nm, eng):
            t = consts.tile([128, EC, n], BF, name=nm)
            eng.dma_start(out=t, in_=d.ap().rearrange("(c p) n -> p c n", p=128))
            return t

        wk = load_w(wk_d, D, "wk_sb", nc.sync)
        wvb = load_w(wvb_d, 2 * D, "wvb_sb", nc.sync)
        wa1 = load_w(wa1_d, 32, "wa1_sb", nc.sync)
        wa2 = consts.tile([32, D], BF)
        nc.sync.dma_start(out=wa2, in_=wa2_d.ap())
        ws1 = load_w(ws1_d, 32, "ws1_sb", nc.scalar)
        ws2 = consts.tile([32, D], BF)
        nc.scalar.dma_start(out=ws2, in_=ws2_d.ap())
        wq = load_w(wq_d, D, "wq_sb", nc.scalar)
        wo2 = consts.tile([128, D], BF)   # wo duplicated into both halves
        nc.scalar.dma_start(out=wo2[0:64, :], in_=wo_d.ap())
        nc.scalar.dma_start(out=wo2[64:128, :], in_=wo_d.ap())
        bias_sb = {}
        for name, t in bias_d.items():
            n = t.shape[1]
            bt = consts.tile([1, n], BF, name=f"{name}_sb")
            nc.scalar.dma_start(out=bt, in_=t.ap())
            bias_sb[name] = bt

        def bias_mm(psum, name):
            """Add per-column bias b[1, n] to psum accumulation via K=1 matmul."""
            if name not in bias_sb:
                return False
            nc.tensor.matmul(psum, ones_row[:, : psum.shape[0]], bias_sb[name],
                             start=False, stop=True)
            return True

        def biasT_mm(psum, name):
            """Add per-row bias (transposed layouts): psum[r, m] += b[r]."""
            if name not in bias_sb:
                return False
            nc.tensor.matmul(psum, bias_sb[name], ones_row[:, : psum.free_size()],
                             start=False, stop=True)
            return True

        # persistent SBUF tensors
        k2o = persist.tile([128, KT, D + 1], BF)    # k/|k| plus a ones column
        nc.vector.memset(k2o[:, :, D:D + 1], 1.0)
        v1 = persist.tile([128, KT, D], BF)
        ss_k = persist.tile([128, KT], F32)
        rs_k = persist.tile([128, KT], F32)
        hq_sb = persist.tile([128, EC, R], BF)
        hs_sb = persist.tile([128, EC, R], BF)
        # query-half-stacked tensors: rows 0:64 queries 0:512, 64:128 rest
        qT_sb = persist.tile([128, 512], BF)
        sqq_sb = persist.tile([128, 512], BF)
        tsc = persist.tile([128, 512], BF)
        P1sb = persist.tile([128, 512], BF)
        sqP1 = persist.tile([128, 512], BF)
        yTA = persist.tile([128, 512], BF)
        yTB = persist.tile([128, 512], BF)
        Gfull = persist.tile([128, D], BF)          # G in rows 0:64 AND 64:128
        P0row = persist.tile([1, D], BF)
        P0col_b = persist.tile([128, 1], BF)        # P0 dup'd in both halves
        P0col_f = persist.tile([128, 1], F32)
        sqP0 = persist.tile([128, 1], BF)
        c0_c = persist.tile([128, 1], F32)
        ssq_c = persist.tile([128, RT], F32)
        rq_c = persist.tile([128, RT], F32)
        rms_c = persist.tile([128, RT], F32)
        rmsq_c = persist.tile([128, RT], F32)
        out_sb = persist.tile([128, RT, D], F32)

        def rsqrt_newton(eng, dst, src, pool, iters=1):
            """dst = 1/sqrt(src) via Quake bit-trick + Newton on `eng`.
            src: [128, n] f32, n <= KT."""
            n = src.shape[-1]
            I32 = mybir.dt.int32
            i1 = pool.tile([128, KT], I32, tag="rqi", name="rqi")[:, :n]
            eng.tensor_scalar(out=i1, in0=src.bitcast(I32), scalar1=1,
                              scalar2=None, op0=ALU.arith_shift_right)
            x0 = pool.tile([128, KT], F32, tag="rqx", name="rqx")[:, :n]
            eng.tensor_tensor(out=x0.bitcast(I32), in0=magic_i[:, :n],
                              in1=i1, op=ALU.subtract)
            h = pool.tile([128, KT], F32, tag="rqh", name="rqh")[:, :n]
            eng.tensor_scalar_mul(h, src, 0.5)
            cur = x0
            for it in range(iters):
                t = pool.tile([128, KT], F32, tag="rqt", name="rqt")[:, :n]
                eng.tensor_mul(t, cur, cur)
                eng.tensor_mul(t, t, h)
                eng.tensor_scalar(out=t, in0=t, scalar1=-1.0,
                                  scalar2=None, op0=ALU.mult)
                eng.tensor_scalar(out=t, in0=t, scalar1=1.5,
                                  scalar2=None, op0=ALU.add)
                dst_it = dst if it == iters - 1 else pool.tile(
                    [128, KT], F32, tag="rqn", name="rqn")[:, :n]
                eng.tensor_mul(dst_it, cur, t)
                cur = dst_it

        with tc.tile_pool(name="loadk", bufs=4) as loadk, \
             tc.tile_pool(name="loadv", bufs=4) as loadv, \
             tc.tile_pool(name="sig", bufs=4) as sig, \
             tc.tile_pool(name="psA", bufs=2, space="PSUM") as psA, \
             tc.tile_pool(name="psP1", bufs=1, space="PSUM") as psP1, \
             tc.tile_pool(name="psG", bufs=1, space="PSUM") as psG, \
             tc.tile_pool(name="psC", bufs=1, space="PSUM") as psC, \
             tc.tile_pool(name="psPo", bufs=2, space="PSUM") as psPo:

            G_ps = psG.tile([128, D], F32, name="G_ps")
            pcols = psC.tile([128, RT, 3], F32, tag="pc", name="pcols")

            # ================= k/v phase: 4 blocks of 512 keys =================
            hkT_src = hkT_d.ap().rearrange("(c p) r -> p c r", p=128)
            hvT_src = hvT_d.ap().rearrange("(c p) r -> p c r", p=128)
            for blk in range(4):
                ks = slice(blk * 512, (blk + 1) * 512)
                hkb = loadk.tile([128, EC, 512], BF, tag="hk", name="hkb")
                nc.gpsimd.dma_start(out=hkb, in_=hkT_src[:, :, ks])
                hvb = loadv.tile([128, EC, 512], BF, tag="hv", name="hvb")
                nc.sync.dma_start(out=hvb, in_=hvT_src[:, :, ks])
                if blk == 0:
                    nc.sync.dma_start(
                        out=hq_sb,
                        in_=hqT_d.ap().rearrange("(c p) r -> p c r", p=128))
                    nc.sync.dma_start(
                        out=hs_sb,
                        in_=hsT_d.ap().rearrange("(c p) r -> p c r", p=128))

                # ---- k projection (row-major) + silu + |k| ----
                pk = psA.tile([128, 4, D], F32, tag="proj", name="pk")
                for t in range(4):
                    for c in range(EC):
                        nc.tensor.matmul(
                            pk[:, t, :], hkb[:, c, t * 128:(t + 1) * 128],
                            wk[:, c, :], start=(c == 0),
                            stop=(c == EC - 1 and not has_bias["bk"]))
                    bias_mm(pk[:, t, :], "bk")
                ktan = sig.tile([128, 4, D], BF, tag="sig", name="ktan")
                nc.scalar.activation(ktan, pk, AF.Tanh)
                kf = sig.tile([128, 4, D], BF, tag="kf", name="kf")
                nc.vector.scalar_tensor_tensor(
                    out=kf, in0=ktan, scalar=1.0, in1=pk,
                    op0=ALU.add, op1=ALU.mult)
                ksq = sig.tile([128, 4, D], BF, tag="ksq", name="ksq")
                g = blk * 4
                for t in range(4):
                    nc.scalar.activation(ksq[:, t, :], kf[:, t, :], AF.Square,
                                         accum_out=ss_k[:, g + t:g + t + 1])
                rsqrt_newton(nc.vector, rs_k[:, g:g + 4], ss_k[:, g:g + 4],
                             sig, iters=1)
                for t in range(4):
                    nc.scalar.activation(k2o[:, g + t, :D], kf[:, t, :],
                                         AF.Copy, scale=rs_k[:, g + t:g + t + 1])

                # ---- v | beta projection + silu ----
                vbt = sig.tile([128, 4, 2 * D], BF, tag="vbt", name="vbt")
                vf = sig.tile([128, 4, D], BF, tag="vf", name="vf")
                for u in range(2):
                    pvb = psA.tile([128, 2, 2 * D], F32, tag="proj", name="pvb")
                    for hh in range(2):
                        t = 2 * u + hh
                        for c in range(EC):
                            nc.tensor.matmul(
                                pvb[:, hh, :],
                                hvb[:, c, t * 128:(t + 1) * 128],
                                wvb[:, c, :], start=(c == 0),
                                stop=(c == EC - 1 and not has_bias["bvb"]))
                        bias_mm(pvb[:, hh, :], "bvb")
                    nc.scalar.activation(vbt[:, 2 * u:2 * u + 2, :], pvb, AF.Tanh)
                    nc.vector.scalar_tensor_tensor(
                        out=vf[:, 2 * u:2 * u + 2, :],
                        in0=vbt[:, 2 * u:2 * u + 2, :D], scalar=1.0,
                        in1=pvb[:, :, :D], op0=ALU.add, op1=ALU.mult)

                # ---- alpha: a1T (weight-stationary) then a2 (row-major) ----
                pa1 = psA.tile([32, 512], F32, tag="proj", name="pa1")
                for c in range(EC):
                    nc.tensor.matmul(pa1, wa1[:, c, :], hvb[:, c, :],
                                     start=(c == 0),
                                     stop=(c == EC - 1 and not has_bias["ba1"]))
                biasT_mm(pa1, "ba1")
                a1T = sig.tile([32, 512], BF, tag="a1T", name="a1T")
                nc.vector.tensor_copy(a1T, pa1)
                pa2 = psA.tile([128, 4, D], F32, tag="proj", name="pa2")
                for t in range(4):
                    nc.tensor.matmul(pa2[:, t, :],
                                     a1T[:, t * 128:(t + 1) * 128], wa2,
                                     start=True, stop=not has_bias["ba2"])
                    bias_mm(pa2[:, t, :], "ba2")
                alf = sig.tile([128, 4, D], BF, tag="sig", name="alf")
                nc.scalar.activation(alf, pa2, AF.Tanh)
                # v1 = vf*(alf+1) + (vbt_beta+1)   (= 2*(v*alpha+beta))
                t1 = sig.tile([128, 4, D], BF, tag="t1", name="t1")
                nc.vector.scalar_tensor_tensor(
                    out=t1, in0=alf, scalar=1.0, in1=vf,
                    op0=ALU.add, op1=ALU.mult)
                nc.vector.scalar_tensor_tensor(
                    out=v1[:, g:g + 4, :], in0=vbt[:, :, D:], scalar=1.0,
                    in1=t1, op0=ALU.add, op1=ALU.add)

                # ---- G accumulation: G[0:64] += k2^T v1, G[64] += sum v1 ----
                for t in range(4):
                    jt = g + t
                    nc.tensor.matmul(G_ps[0:65, :], k2o[:, jt, :],
                                     v1[:, jt, :], start=(jt == 0),
                                     stop=(jt == KT - 1))

                if blk == 1:
                    # ======= query path (overlaps kv blocks 2-3) =======
                    pq = psA.tile([128, 512], F32, tag="proj", name="pq")
                    for h in range(2):
                        for c in range(EC):
                            nc.tensor.matmul(
                                pq[64 * h:64 * h + 64, :], wq[:, c, :],
                                hq_sb[:, c, h * 512:(h + 1) * 512],
                                start=(c == 0),
                                stop=(c == EC - 1 and not has_bias["bq"]))
                        biasT_mm(pq[64 * h:64 * h + 64, :], "bq")
                    qtan = sig.tile([128, 512], BF, tag="w512", name="qtan")
                    nc.scalar.activation(qtan, pq, AF.Tanh)
                    nc.vector.scalar_tensor_tensor(
                        out=qT_sb, in0=qtan, scalar=1.0, in1=pq,
                        op0=ALU.add, op1=ALU.mult)
                    nc.scalar.activation(sqq_sb, qT_sb, AF.Square)
                    for tt in range(RT):
                        h, cc = tt // 4, tt % 4
                        hp = slice(64 * h, 64 * h + 64)
                        cs = slice(cc * 128, (cc + 1) * 128)
                        nc.tensor.matmul(pcols[:, tt, 0:1], sqq_sb[hp, cs],
                                         ones128c[hp, :], start=True, stop=True)
                    nc.vector.tensor_copy(ssq_c, pcols[:, :, 0])
                    ssq64 = sig.tile([128, RT], F32, tag="ep", name="ssq64")
                    nc.vector.tensor_scalar_mul(ssq64, ssq_c, 64.0)
                    rsqrt_newton(nc.vector, rq_c, ssq64, sig, iters=1)

                    # ======= shortcut path =======
                    ps2 = psA.tile([128, 512], F32, tag="proj", name="ps2")
                    for h in range(2):
                        ps1 = psPo.tile([32, 512], F32, tag="po", name="ps1")
                        for c in range(EC):
                            nc.tensor.matmul(
                                ps1, ws1[:, c, :],
                                hs_sb[:, c, h * 512:(h + 1) * 512],
                                start=(c == 0),
                                stop=(c == EC - 1 and not has_bias["bs1"]))
                        biasT_mm(ps1, "bs1")
                        s1T = sig.tile([32, 512], BF, tag="a1T", name="s1T")
                        nc.vector.tensor_copy(s1T, ps1)
                        nc.tensor.matmul(ps2[64 * h:64 * h + 64, :], ws2, s1T,
                                         start=True, stop=not has_bias["bs2"])
                        biasT_mm(ps2[64 * h:64 * h + 64, :], "bs2")
                    nc.scalar.activation(tsc, ps2, AF.Tanh)

            # ================= tail =================
            # early ACT table switch (tanh set -> sqrt set) behind PE work
            sqd = sig.tile([1, 1], F32, tag="sqd", name="sqd")
            nc.scalar.activation(sqd, onef, AF.Sqrt)

            # G / P0 evacuation + duplication into partitions 64:128
            nc.vector.tensor_copy(Gfull[0:64, :], G_ps[0:64, :])
            nc.vector.tensor_copy(P0row, G_ps[64:65, :])
            gd_ps = psC.tile([128, D], F32, tag="tiny", name="gd_ps")
            nc.tensor.matmul(gd_ps[64:128, :], ident64, Gfull[0:64, :],
                             start=True, stop=True)
            nc.vector.tensor_copy(Gfull[64:128, :], gd_ps[64:128, :])
            p0c_ps = psC.tile([128, 1], BF, tag="tiny", name="p0c_ps")
            nc.tensor.transpose(p0c_ps[0:64, :], P0row, ones128c[0:1, :])
            nc.tensor.transpose(p0c_ps[64:128, :], P0row, ones128c[0:1, :])
            nc.vector.tensor_copy(P0col_b, p0c_ps)
            nc.vector.tensor_copy(P0col_f, p0c_ps)
            nc.scalar.activation(sqP0, P0col_b, AF.Square)
            c0_ps = psC.tile([128, 1], F32, tag="tiny", name="c0_ps")
            nc.tensor.matmul(c0_ps, ones64x128, sqP0[0:64, :],
                             start=True, stop=True)
            nc.vector.tensor_copy(c0_c, c0_ps)

            # P1 = G^T qT (both halves via quadrants)
            pP1 = psP1.tile([128, 512], F32, tag="p1", name="pP1")
            nc.tensor.matmul(pP1[0:64, :], Gfull[0:64, :], qT_sb[0:64, :],
                             start=True, stop=True)
            nc.tensor.matmul(pP1[64:128, :], Gfull[64:128, :], qT_sb[64:128, :],
                             start=True, stop=True)
            # yTB = (tsc+1) * P1 ; yTA = (tsc+1) * P0 = tsc*P0 + P0
            nc.vector.scalar_tensor_tensor(
                out=yTB, in0=tsc, scalar=1.0, in1=pP1,
                op0=ALU.add, op1=ALU.mult)
            nc.scalar.activation(yTA, tsc, AF.Identity,
                                 scale=P0col_f, bias=P0col_f)
            nc.scalar.activation(P1sb, pP1, AF.Copy)
            nc.scalar.activation(sqP1, pP1, AF.Square)

            # u / w columns per query tile
            for tt in range(RT):
                h, cc = tt // 4, tt % 4
                hp = slice(64 * h, 64 * h + 64)
                cs = slice(cc * 128, (cc + 1) * 128)
                nc.tensor.matmul(pcols[:, tt, 1:2], P1sb[hp, cs],
                                 P0col_b[hp, :], start=True, stop=True)
                nc.tensor.matmul(pcols[:, tt, 2:3], sqP1[hp, cs],
                                 ones128c[hp, :], start=True, stop=True)
            uw = sig.tile([128, RT, 2], F32, tag="uw", name="uw")
            nc.vector.tensor_copy(uw, pcols[:, :, 1:3])

            # ms*64 = c0 + 2*rq*u + rq^2*w ; rms = rsqrt(ms + eps)
            tA = sig.tile([128, RT], F32, tag="ep", name="tA")
            nc.vector.tensor_mul(tA, rq_c, uw[:, :, 0])
            tB = sig.tile([128, RT], F32, tag="ep2", name="tB")
            nc.vector.tensor_mul(tB, rq_c, rq_c)
            nc.vector.tensor_mul(tB, tB, uw[:, :, 1])
            nc.vector.scalar_tensor_tensor(
                out=tB, in0=tA, scalar=2.0, in1=tB,
                op0=ALU.mult, op1=ALU.add)
            nc.vector.tensor_scalar(out=tB, in0=tB, scalar1=c0_c,
                                    scalar2=None, op0=ALU.add)
            srt = sig.tile([128, RT], F32, tag="ep3", name="srt")
            nc.scalar.activation(srt, tB, AF.Sqrt, scale=1.0 / 64, bias=eps128)
            nc.vector.reciprocal(rms_c, srt)
            nc.vector.tensor_mul(rmsq_c, rms_c, rq_c)

            # final Wo projections + per-query scaling
            for tt in range(RT):
                h, cc = tt // 4, tt % 4
                hp = slice(64 * h, 64 * h + 64)
                cs = slice(cc * 128, (cc + 1) * 128)
                po2 = psPo.tile([128, 2, D], F32, tag="po", name="po2")
                nc.tensor.matmul(po2[:, 0, :], yTA[hp, cs], wo2[hp, :],
                                 start=True, stop=not has_bias["bo"])
                bias_mm(po2[:, 0, :], "bo")
                nc.tensor.matmul(po2[:, 1, :], yTB[hp, cs], wo2[hp, :],
                                 start=True, stop=True)
                tmp = sig.tile([128, D], F32, tag="tmp", name="tmp")
                nc.scalar.activation(tmp, po2[:, 0, :], AF.Copy,
                                     scale=rms_c[:, tt:tt + 1])
                nc.vector.scalar_tensor_tensor(
                    out=out_sb[:, tt, :], in0=po2[:, 1, :],
                    scalar=rmsq_c[:, tt:tt + 1], in1=tmp,
                    op0=ALU.mult, op1=ALU.add)
                if tt == 3 or tt == RT - 1:
                    csl = slice(0, 4) if tt == 3 else slice(4, 8)
                    nc.sync.dma_start(
                        out=out_d.ap().rearrange("(t p) n -> p t n", p=128)[
                            :, csl, :],
                        in_=out_sb[:, csl, :],
                    )

    nc.compile()
    return nc


_CACHED = None


def kernel(**inputs):
    global LAST, _CACHED
    inp = {k: np.asarray(v) for k, v in inputs.items()}

    bias_map = {"bq": "bq", "bk": "bk", "ba1": "ba1", "ba2": "ba2",
                "bs1": "bs1", "bs2": "bs2", "bo": "bo"}
    has_bias = {k: bool(np.any(inp[v])) for k, v in bias_map.items()}
    has_bias["bvb"] = bool(np.any(inp["bv"]) or np.any(inp["bb"]))

    key = tuple(sorted(has_bias.items()))
    if _CACHED is None or _CACHED[0] != key:
        _CACHED = (key, _build(has_bias))
    nc = _CACHED[1]

    bf = lambda x: np.ascontiguousarray(x.astype(BF16))
    bfT = lambda x: np.ascontiguousarray(x.astype(BF16).T)
    # Gate pre-activations are halved on the host so sigmoid(x)=0.5*tanh(x/2)+0.5
    # and silu(x)=x*sigmoid(x) reduce to tanh + one scalar_tensor_tensor op.
    # The resulting global factor 2 on v1/attn cancels in rmsnorm; the factor 2
    # from the shortcut gate is folded into Wo (with g_rms).
    wo_fold = 0.5 * inp["g_rms"][:, None] * inp["Wo"]
    weights = {
        "wq": bf(0.5 * inp["Wq"]), "wk": bf(0.5 * inp["Wk"]),
        "wvb": bf(0.5 * np.concatenate([inp["Wv"], inp["Wb"]], axis=1)),
        "wa1": bf(inp["Wa1"]), "ws1": bf(inp["Ws1"]),
        "wa2": bf(0.5 * inp["Wa2"]), "ws2": bf(0.5 * inp["Ws2"]),
        "wo": bf(wo_fold),
    }
    if has_bias["bq"]:
        weights["bq"] = bf(0.5 * inp["bq"][None, :])
    if has_bias["bk"]:
        weights["bk"] = bf(0.5 * inp["bk"][None, :])
    if has_bias["bvb"]:
        weights["bvb"] = bf(0.5 * np.concatenate([inp["bv"], inp["bb"]])[None, :])
    if has_bias["ba1"]:
        weights["ba1"] = bf(inp["ba1"][None, :])
    if has_bias["ba2"]:
        weights["ba2"] = bf(0.5 * inp["ba2"][None, :])
    if has_bias["bs1"]:
        weights["bs1"] = bf(inp["bs1"][None, :])
    if has_bias["bs2"]:
        weights["bs2"] = bf(0.5 * inp["bs2"][None, :])
    if has_bias["bo"]:
        weights["bo"] = bf(inp["bo"][None, :])

    in_maps = []
    for c in range(NCORES):
        b, h = c // 2, c % 2
        m = dict(weights)
        m["hqT"] = bfT(inp["hidden_query"][b, h * R:(h + 1) * R])
        m["hkT"] = bfT(inp["hidden_key"][b])
        m["hvT"] = bfT(inp["hidden_value"][b])
        m["hsT"] = bfT(inp["hidden_shortcut"][b, h * R:(h + 1) * R])
        in_maps.append(m)

    LAST = run_bass_kernel_spmd(nc, in_maps, core_ids=list(range(NCORES)))

    out = np.empty((B, L, D), np.float32)
    for c in range(NCORES):
        b, h = c // 2, c % 2
        out[b, h * R:(h + 1) * R] = LAST.results[c]["out"]
    return out


if __name__ == "__main__":
    rng = np.random.default_rng(0)
    fake = {}
    fake["hidden_query"] = rng.standard_normal((B, L, E), dtype=np.float32)
    fake["hidden_key"] = rng.standard_normal((B, L, E), dtype=np.float32)
    fake["hidden_value"] = rng.standard_normal((B, L, E), dtype=np.float32)
    fake["hidden_shortcut"] = rng.standard_normal((B, L, E), dtype=np.float32)
    for n, s in [("Wq", (E, D)), ("Wk", (E, D)), ("Wv", (E, D)), ("Wa1", (E, 32)),
                 ("Wa2", (32, D)), ("Wb", (E, D)), ("Ws1", (E, 32)), ("Ws2", (32, D)),
                 ("Wo", (D, D))]:
        fake[n] = rng.standard_normal(s, dtype=np.float32) * 0.05
    for n, s in [("bq", D), ("bk", D), ("bv", D), ("ba1", 32), ("ba2", D),
                 ("bb", D), ("bs1", 32), ("bs2", D), ("bo", D)]:
        fake[n] = np.zeros(s, np.float32)
    fake["g_rms"] = np.ones(D, np.float32)
    o = kernel(**fake)
    print("ran:", o.shape, o.dtype, np.abs(o).max())


# revision 15
# speedup vs baseline: 1.5398x; 1.1893x over previous
"""Trainium2 Bass kernel for gated single-head attention (B=4, L=2048, E=512, D=64).

Sharding: data-parallel over 8 cores; core c handles batch b=c//2, query-row
half h=c%2 (1024 query rows). hk/hv are processed per-core for the full batch.

Math restructuring (validated in numpy against the jax reference):
  - q,k are L2-normalized so scores s = (q^.k^)/8 lie in [-1/8, 1/8]; softmax
    exp is linearized: e = 1 + s (rel err 6e-6 after rmsnorm, which cancels
    the near-uniform quadratic term). The attention then COLLAPSES to a
    64x64 bilinear form:
        attn[i] = P0 + rs_q_i * (G^T q_i),
        G = sum_j (k_j/|k_j|) (x) v1_j   [64x64],  P0 = sum_j v1_j.
    No 2048x1024 score matrix, no exp, no per-score evacuation.
  - rs_q (1/(8|q_i|)) and the rmsnorm scale are per-query; both are applied
    AFTER the final Wo projection (queries land on partitions there), using
    rmsnorm(P0 + r*P1) algebra: ms*64 = c0 + 2*r*u + r^2*w with
    c0 = |P0|^2, u = P0.P1_i, w = |P1_i|^2 - all computed by tiny matmuls.
  - inputs are pre-transposed AND pre-cast to bf16 on the host: zero PE
    transposes for the projections, and half the HBM traffic.
  - the query path is stacked two 512-query halves on 128 partitions
    (G / P0 / wo duplicated into partitions 64:128, matmuls use quadrant
    tile positions) so every elementwise op runs at full 128-lane rate.
  - gates go through Tanh (sigma(x) = tanh(x/2)/2 + 1/2, the 1/2s folded
    into host-side weights); in-phase rsqrts are DVE/Pool bit-trick Newton;
    the tail switches the ACT table once (dummy Sqrt) and uses
    Sqrt + vector.reciprocal.
Compute dtype bf16, accumulation f32 in PSUM.
"""

import os
import sys

import numpy as np

try:
    import concourse.bass as bass
except ImportError:  # staged container path
    sys.path.insert(0, "/opt/trn_rl_repo")
    import concourse.bass as bass

import ml_dtypes
from contextlib import ExitStack

import concourse.bacc as bacc
import concourse.tile as tile
from concourse import mybir
from concourse.bass_utils import run_bass_kernel_spmd
from concourse.masks import make_identity

BF16 = ml_dtypes.bfloat16
F32 = mybir.dt.float32
BF = mybir.dt.bfloat16
AF = mybir.ActivationFunctionType
ALU = mybir.AluOpType

B, L, E, D = 4, 2048, 512, 64
NCORES = 8
R = L // 2          # 1024 query rows per core
RT = R // 128       # 8 query m-tiles per core
KT = L // 128       # 16 kv m-tiles per core
EC = E // 128       # 4 contraction chunks
EPS_RMS = 4e-6      # 1e-6 * 4 (v1 carries a global factor 2)

LAST = None  # BassKernelResults of the most recent run (for test harness)


def _build(has_bias):
    """Build the per-core SPMD program. has_bias: dict of bool flags."""
    nc = bacc.Bacc(
        "TRN2",
        target_bir_lowering=False,
        debug=False,
        enable_asserts=False,
        num_devices=NCORES,
    )

    hqT_d = nc.dram_tensor("hqT", [E, R], BF, kind="ExternalInput")
    hkT_d = nc.dram_tensor("hkT", [E, L], BF, kind="ExternalInput")
    hvT_d = nc.dram_tensor("hvT", [E, L], BF, kind="ExternalInput")
    hsT_d = nc.dram_tensor("hsT", [E, R], BF, kind="ExternalInput")
    wq_d = nc.dram_tensor("wq", [E, D], BF, kind="ExternalInput")
    wk_d = nc.dram_tensor("wk", [E, D], BF, kind="ExternalInput")
    wvb_d = nc.dram_tensor("wvb", [E, 2 * D], BF, kind="ExternalInput")
    wa1_d = nc.dram_tensor("wa1", [E, 32], BF, kind="ExternalInput")
    ws1_d = nc.dram_tensor("ws1", [E, 32], BF, kind="ExternalInput")
    wa2_d = nc.dram_tensor("wa2", [32, D], BF, kind="ExternalInput")
    ws2_d = nc.dram_tensor("ws2", [32, D], BF, kind="ExternalInput")
    wo_d = nc.dram_tensor("wo", [D, D], BF, kind="ExternalInput")
    bias_d = {}
    for name, n in [("bq", D), ("bk", D), ("bvb", 2 * D), ("ba1", 32),
                    ("ba2", D), ("bs1", 32), ("bs2", D), ("bo", D)]:
        if has_bias[name]:
            bias_d[name] = nc.dram_tensor(name, [1, n], BF, kind="ExternalInput")
    out_d = nc.dram_tensor("out", [R, D], F32, kind="ExternalOutput")

    with tile.TileContext(nc) as tc, ExitStack() as ctx:
        consts = ctx.enter_context(tc.tile_pool(name="consts", bufs=1))
        persist = ctx.enter_context(tc.tile_pool(name="persist", bufs=1))

        ones128c = consts.tile([128, 1], BF)
        nc.vector.memset(ones128c, 1.0)
        ones64x128 = consts.tile([64, 128], BF)
        nc.vector.memset(ones64x128, 1.0)
        onef = consts.tile([1, 1], F32)
        nc.vector.memset(onef, 1.0)
        eps128 = consts.tile([128, 1], F32)
        nc.vector.memset(eps128, EPS_RMS)
        ident64 = consts.tile([64, 64], BF)
        make_identity(nc, ident64)
        magic_i = consts.tile([128, KT], mybir.dt.int32)
        nc.vector.memset(magic_i, 0x5F3759DF)
        any_bias = any(has_bias.values())
        if any_bias:
            ones_row = consts.tile([1, 512], BF)
            nc.vector.memset(ones_row, 1.0)

        # --- weights: kv-path weights early on sync; the rest on scalar ---
        def load_w(d, n, nm, eng):
            t = consts.tile([128, EC, n], BF, name=nm)
            eng.dma_start(out=t, in_=d.ap().rearrange("(c p) n -> p c n", p=128))
            return t

        wk = load_w(wk_d, D, "wk_sb", nc.sync)
        wvb = load_w(wvb_d, 2 * D, "wvb_sb", nc.sync)
        wa1 = load_w(wa1_d, 32, "wa1_sb", nc.scalar)
        wa2 = consts.tile([32, D], BF)
        nc.scalar.dma_start(out=wa2, in_=wa2_d.ap())
        ws1 = load_w(ws1_d, 32, "ws1_sb", nc.scalar)
        ws2 = consts.tile([32, D], BF)
        nc.scalar.dma_start(out=ws2, in_=ws2_d.ap())
        wq = load_w(wq_d, D, "wq_sb", nc.scalar)
        wo2 = consts.tile([128, D], BF)   # wo duplicated into both halves
        nc.scalar.dma_start(out=wo2[0:64, :], in_=wo_d.ap())
        nc.scalar.dma_start(out=wo2[64:128, :], in_=wo_d.ap())
        bias_sb = {}
        for name, t in bias_d.items():
            n = t.shape[1]
            bt = consts.tile([1, n], BF, name=f"{name}_sb")
            nc.scalar.dma_start(out=bt, in_=t.ap())
            bias_sb[name] = bt

        def bias_mm(psum, name):
            """Add per-column bias b[1, n] to psum accumulation via K=1 matmul."""
            if name not in bias_sb:
                return False
            nc.tensor.matmul(psum, ones_row[:, : psum.shape[0]], bias_sb[name],
                             start=False, stop=True)
            return True

        def biasT_mm(psum, name):
            """Add per-row bias (transposed layouts): psum[r, m] += b[r]."""
            if name not in bias_sb:
                return False
            nc.tensor.matmul(psum, bias_sb[name], ones_row[:, : psum.free_size()],
                             start=False, stop=True)
            return True

        # persistent SBUF tensors
        k2o = persist.tile([128, KT, D + 1], BF)    # k/|k| plus a ones column
        nc.vector.memset(k2o[:, :, D:D + 1], 1.0)
        v1 = persist.tile([128, KT, D], BF)
        ss_k = persist.tile([128, KT], F32)
        rs_k = persist.tile([128, KT], F32)
        hq_sb = persist.tile([128, EC, R], BF)
        hs_sb = persist.tile([128, EC, R], BF)
        # query-half-stacked tensors: rows 0:64 queries 0:512, 64:128 rest
        qT_sb = persist.tile([128, 512], BF)
        sqq_sb = persist.tile([128, 512], BF)
        tsc = persist.tile([128, 512], BF)
        P1sb = persist.tile([128, 512], BF)
        sqP1 = persist.tile([128, 512], BF)
        yTA = persist.tile([128, 512], BF)
        yTB = persist.tile([128, 512], BF)
        Gfull = persist.tile([128, D], BF)          # G in rows 0:64 AND 64:128
        P0row = persist.tile([1, D], BF)
        P0col_b = persist.tile([128, 1], BF)        # P0 dup'd in both halves
        P0col_f = persist.tile([128, 1], F32)
        sqP0 = persist.tile([128, 1], BF)
        c0_c = persist.tile([128, 1], F32)
        ssq_c = persist.tile([128, RT], F32)
        rq_c = persist.tile([128, RT], F32)
        rms_c = persist.tile([128, RT], F32)
        rmsq_c = persist.tile([128, RT], F32)
        out_sb = persist.tile([128, RT, D], F32)

        def rsqrt_newton(eng, dst, src, pool, iters=1):
            """dst = 1/sqrt(src) via Quake bit-trick + Newton on `eng`.
            src: [128, n] f32, n <= KT."""
            n = src.shape[-1]
            I32 = mybir.dt.int32
            i1 = pool.tile([128, KT], I32, tag="rqi", name="rqi")[:, :n]
            eng.tensor_scalar(out=i1, in0=src.bitcast(I32), scalar1=1,
                              scalar2=None, op0=ALU.arith_shift_right)
            x0 = pool.tile([128, KT], F32, tag="rqx", name="rqx")[:, :n]
            eng.tensor_tensor(out=x0.bitcast(I32), in0=magic_i[:, :n],
                              in1=i1, op=ALU.subtract)
            h = pool.tile([128, KT], F32, tag="rqh", name="rqh")[:, :n]
            eng.tensor_scalar_mul(h, src, 0.5)
            cur = x0
            for it in range(iters):
                t = pool.tile([128, KT], F32, tag="rqt", name="rqt")[:, :n]
                eng.tensor_mul(t, cur, cur)
                eng.tensor_mul(t, t, h)
                eng.tensor_scalar(out=t, in0=t, scalar1=-1.0,
                                  scalar2=None, op0=ALU.mult)
                eng.tensor_scalar(out=t, in0=t, scalar1=1.5,
                                  scalar2=None, op0=ALU.add)
                dst_it = dst if it == iters - 1 else pool.tile(
                    [128, KT], F32, tag="rqn", name="rqn")[:, :n]
                eng.tensor_mul(dst_it, cur, t)
                cur = dst_it

        with tc.tile_pool(name="loadk", bufs=4) as loadk, \
             tc.tile_pool(name="loadv", bufs=4) as loadv, \
             tc.tile_pool(name="sig", bufs=4) as sig, \
             tc.tile_pool(name="psA", bufs=3, space="PSUM") as psA, \
             tc.tile_pool(name="psP1", bufs=1, space="PSUM") as psP1, \
             tc.tile_pool(name="psG", bufs=1, space="PSUM") as psG, \
             tc.tile_pool(name="psC", bufs=1, space="PSUM") as psC, \
             tc.tile_pool(name="psPo", bufs=1, space="PSUM") as psPo:

            G_ps = psG.tile([128, D], F32, name="G_ps")
            pcols = psC.tile([128, RT, 3], F32, tag="pc", name="pcols")

            # ================= k/v phase: 4 blocks of 512 keys =================
            hkT_src = hkT_d.ap().rearrange("(c p) r -> p c r", p=128)
            hvT_src = hvT_d.ap().rearrange("(c p) r -> p c r", p=128)
            for blk in range(4):
                ks = slice(blk * 512, (blk + 1) * 512)
                hkb = loadk.tile([128, EC, 512], BF, tag="hk", name="hkb")
                nc.gpsimd.dma_start(out=hkb, in_=hkT_src[:, :, ks])
                hvb = loadv.tile([128, EC, 512], BF, tag="hv", name="hvb")
                nc.sync.dma_start(out=hvb, in_=hvT_src[:, :, ks])
                if blk == 0:
                    nc.sync.dma_start(
                        out=hq_sb,
                        in_=hqT_d.ap().rearrange("(c p) r -> p c r", p=128))
                    nc.sync.dma_start(
                        out=hs_sb,
                        in_=hsT_d.ap().rearrange("(c p) r -> p c r", p=128))

                # ---- k projection (row-major) + silu + |k| ----
                pk = psA.tile([128, 4, D], F32, tag="proj", name="pk")
                for t in range(4):
                    for c in range(EC):
                        nc.tensor.matmul(
                            pk[:, t, :], hkb[:, c, t * 128:(t + 1) * 128],
                            wk[:, c, :], start=(c == 0),
                            stop=(c == EC - 1 and not has_bias["bk"]))
                    bias_mm(pk[:, t, :], "bk")
                kf = sig.tile([128, 4, D], BF, tag="kf", name="kf")
                nc.scalar.activation(kf, pk, AF.Silu, scale=2.0)
                ksq = sig.tile([128, 4, D], BF, tag="ksq", name="ksq")
                nc.scalar.activation(ksq, kf, AF.Square)
                g = blk * 4
                nc.vector.reduce_sum(
                    ss_k[:, g:g + 4].rearrange("p (a b) -> p a b", b=1),
                    ksq, axis=mybir.AxisListType.X)
                rsqrt_newton(nc.vector, rs_k[:, g:g + 4], ss_k[:, g:g + 4],
                             sig, iters=1)
                for t in range(4):
                    nc.vector.tensor_scalar_mul(
                        k2o[:, g + t, :D], kf[:, t, :], rs_k[:, g + t:g + t + 1])

                # ---- v | beta projection + silu ----
                vbt = sig.tile([128, 4, 2 * D], BF, tag="vbt", name="vbt")
                vf = sig.tile([128, 4, D], BF, tag="vf", name="vf")
                for u in range(2):
                    pvb = psA.tile([128, 2, 2 * D], F32, tag="proj", name="pvb")
                    for hh in range(2):
                        t = 2 * u + hh
                        for c in range(EC):
                            nc.tensor.matmul(
                                pvb[:, hh, :],
                                hvb[:, c, t * 128:(t + 1) * 128],
                                wvb[:, c, :], start=(c == 0),
                                stop=(c == EC - 1 and not has_bias["bvb"]))
                        bias_mm(pvb[:, hh, :], "bvb")
                    nc.scalar.activation(vbt[:, 2 * u:2 * u + 2, D:], pvb[:, :, D:],
                                         AF.Tanh)
                    nc.scalar.activation(vf[:, 2 * u:2 * u + 2, :],
                                         pvb[:, :, :D], AF.Silu, scale=2.0)

                # ---- alpha: a1T (weight-stationary) then a2 (row-major) ----
                pa1 = psA.tile([32, 512], F32, tag="proj", name="pa1")
                for c in range(EC):
                    nc.tensor.matmul(pa1, wa1[:, c, :], hvb[:, c, :],
                                     start=(c == 0),
                                     stop=(c == EC - 1 and not has_bias["ba1"]))
                biasT_mm(pa1, "ba1")
                a1T = sig.tile([32, 512], BF, tag="a1T", name="a1T")
                nc.scalar.activation(a1T, pa1, AF.Copy)
                pa2 = psA.tile([128, 4, D], F32, tag="proj", name="pa2")
                for t in range(4):
                    nc.tensor.matmul(pa2[:, t, :],
                                     a1T[:, t * 128:(t + 1) * 128], wa2,
                                     start=True, stop=not has_bias["ba2"])
                    bias_mm(pa2[:, t, :], "ba2")
                alf = sig.tile([128, 4, D], BF, tag="sig", name="alf")
                nc.scalar.activation(alf, pa2, AF.Tanh)
                # v1 = vf*(alf+1) + (vbt_beta+1)   (= 2*(v*alpha+beta))
                t1 = sig.tile([128, 4, D], BF, tag="t1", name="t1")
                nc.vector.scalar_tensor_tensor(
                    out=t1, in0=alf, scalar=1.0, in1=vf,
                    op0=ALU.add, op1=ALU.mult)
                nc.vector.scalar_tensor_tensor(
                    out=v1[:, g:g + 4, :], in0=vbt[:, :, D:], scalar=1.0,
                    in1=t1, op0=ALU.add, op1=ALU.add)

                # ---- G accumulation: G[0:64] += k2^T v1, G[64] += sum v1 ----
                for t in range(4):
                    jt = g + t
                    nc.tensor.matmul(G_ps[0:65, :], k2o[:, jt, :],
                                     v1[:, jt, :], start=(jt == 0),
                                     stop=(jt == KT - 1))

                if blk == 1:
                    # ======= query path (overlaps kv blocks 2-3) =======
                    pq = psA.tile([128, 512], F32, tag="proj", name="pq")
                    for h in range(2):
                        for c in range(EC):
                            nc.tensor.matmul(
                                pq[64 * h:64 * h + 64, :], wq[:, c, :],
                                hq_sb[:, c, h * 512:(h + 1) * 512],
                                start=(c == 0),
                                stop=(c == EC - 1 and not has_bias["bq"]))
                        biasT_mm(pq[64 * h:64 * h + 64, :], "bq")
                    nc.scalar.activation(qT_sb, pq, AF.Silu, scale=2.0)
                    nc.scalar.activation(sqq_sb, qT_sb, AF.Square)
                    for tt in range(RT):
                        h, cc = tt // 4, tt % 4
                        hp = slice(64 * h, 64 * h + 64)
                        cs = slice(cc * 128, (cc + 1) * 128)
                        nc.tensor.matmul(pcols[:, tt, 0:1], sqq_sb[hp, cs],
                                         ones128c[hp, :], start=True, stop=True)
                    nc.vector.tensor_copy(ssq_c, pcols[:, :, 0])
                    ssq64 = sig.tile([128, RT], F32, tag="ep", name="ssq64")
                    nc.vector.tensor_scalar_mul(ssq64, ssq_c, 64.0)
                    rsqrt_newton(nc.vector, rq_c, ssq64, sig, iters=1)

                    # ======= shortcut path =======
                    ps2 = psA.tile([128, 512], F32, tag="proj", name="ps2")
                    for h in range(2):
                        ps1 = psPo.tile([32, 512], F32, tag="po", name="ps1")
                        for c in range(EC):
                            nc.tensor.matmul(
                                ps1, ws1[:, c, :],
                                hs_sb[:, c, h * 512:(h + 1) * 512],
                                start=(c == 0),
                                stop=(c == EC - 1 and not has_bias["bs1"]))
                        biasT_mm(ps1, "bs1")
                        s1T = sig.tile([32, 512], BF, tag="a1T", name="s1T")
                        nc.vector.tensor_copy(s1T, ps1)
                        nc.tensor.matmul(ps2[64 * h:64 * h + 64, :], ws2, s1T,
                                         start=True, stop=not has_bias["bs2"])
                        biasT_mm(ps2[64 * h:64 * h + 64, :], "bs2")
                    nc.scalar.activation(tsc, ps2, AF.Tanh)

            # ================= tail =================
            # early ACT table switch (tanh set -> sqrt set) behind PE work
            sqd = sig.tile([1, 1], F32, tag="sqd", name="sqd")
            nc.scalar.activation(sqd, onef, AF.Sqrt)

            # G / P0 evacuation + duplication into partitions 64:128
            nc.vector.tensor_copy(Gfull[0:64, :], G_ps[0:64, :])
            nc.vector.tensor_copy(P0row, G_ps[64:65, :])
            gd_ps = psC.tile([128, D], F32, tag="tiny", name="gd_ps")
            nc.tensor.matmul(gd_ps[64:128, :], ident64, Gfull[0:64, :],
                             start=True, stop=True)
            nc.vector.tensor_copy(Gfull[64:128, :], gd_ps[64:128, :])
            p0c_ps = psC.tile([128, 1], BF, tag="tiny", name="p0c_ps")
            nc.tensor.transpose(p0c_ps[0:64, :], P0row, ones128c[0:1, :])
            nc.tensor.transpose(p0c_ps[64:128, :], P0row, ones128c[0:1, :])
            nc.vector.tensor_copy(P0col_b, p0c_ps)
            nc.vector.tensor_copy(P0col_f, p0c_ps)
            nc.scalar.activation(sqP0, P0col_b, AF.Square)
            c0_ps = psC.tile([128, 1], F32, tag="tiny", name="c0_ps")
            nc.tensor.matmul(c0_ps, ones64x128, sqP0[0:64, :],
                             start=True, stop=True)
            nc.vector.tensor_copy(c0_c, c0_ps)

            # P1 = G^T qT (both halves via quadrants)
            pP1 = psP1.tile([128, 512], F32, tag="p1", name="pP1")
            nc.tensor.matmul(pP1[0:64, :], Gfull[0:64, :], qT_sb[0:64, :],
                             start=True, stop=True)
            nc.tensor.matmul(pP1[64:128, :], Gfull[64:128, :], qT_sb[64:128, :],
                             start=True, stop=True)
            # yTB = (tsc+1) * P1 ; yTA = (tsc+1) * P0 = tsc*P0 + P0
            nc.vector.scalar_tensor_tensor(
                out=yTB, in0=tsc, scalar=1.0, in1=pP1,
                op0=ALU.add, op1=ALU.mult)
            nc.scalar.activation(yTA, tsc, AF.Identity,
                                 scale=P0col_f, bias=P0col_f)
            nc.scalar.activation(P1sb, pP1, AF.Copy)
            nc.scalar.activation(sqP1, pP1, AF.Square)

            # u / w columns per query tile
            for tt in range(RT):
                h, cc = tt // 4, tt % 4
                hp = slice(64 * h, 64 * h + 64)
                cs = slice(cc * 128, (cc + 1) * 128)
                nc.tensor.matmul(pcols[:, tt, 1:2], P1sb[hp, cs],
                                 P0col_b[hp, :], start=True, stop=True)
                nc.tensor.matmul(pcols[:, tt, 2:3], sqP1[hp, cs],
                                 ones128c[hp, :], start=True, stop=True)
            uw = sig.tile([128, RT, 2], F32, tag="uw", name="uw")
            nc.vector.tensor_copy(uw, pcols[:, :, 1:3])

            # ms*64 = c0 + 2*rq*u + rq^2*w ; rms = rsqrt(ms + eps)
            tA = sig.tile([128, RT], F32, tag="ep", name="tA")
            nc.vector.tensor_mul(tA, rq_c, uw[:, :, 0])
            tB = sig.tile([128, RT], F32, tag="ep2", name="tB")
            nc.vector.tensor_mul(tB, rq_c, rq_c)
            nc.vector.tensor_mul(tB, tB, uw[:, :, 1])
            nc.vector.scalar_tensor_tensor(
                out=tB, in0=tA, scalar=2.0, in1=tB,
                op0=ALU.mult, op1=ALU.add)
            nc.vector.tensor_scalar(out=tB, in0=tB, scalar1=c0_c,
                                    scalar2=None, op0=ALU.add)
            srt = sig.tile([128, RT], F32, tag="ep3", name="srt")
            nc.scalar.activation(srt, tB, AF.Sqrt, scale=1.0 / 64, bias=eps128)
            nc.vector.reciprocal(rms_c, srt)
            nc.vector.tensor_mul(rmsq_c, rms_c, rq_c)

            # final Wo projections + per-query scaling
            for tt in range(RT):
                h, cc = tt // 4, tt % 4
                hp = slice(64 * h, 64 * h + 64)
                cs = slice(cc * 128, (cc + 1) * 128)
                po2 = psPo.tile([128, 2, D], F32, tag="po", name="po2")
                nc.tensor.matmul(po2[:, 0, :], yTA[hp, cs], wo2[hp, :],
                                 start=True, stop=not has_bias["bo"])
                bias_mm(po2[:, 0, :], "bo")
                nc.tensor.matmul(po2[:, 1, :], yTB[hp, cs], wo2[hp, :],
                                 start=True, stop=True)
                tmp = sig.tile([128, D], F32, tag="tmp", name="tmp")
                nc.scalar.activation(tmp, po2[:, 0, :], AF.Copy,
                                     scale=rms_c[:, tt:tt + 1])
                nc.vector.scalar_tensor_tensor(
                    out=out_sb[:, tt, :], in0=po2[:, 1, :],
                    scalar=rmsq_c[:, tt:tt + 1], in1=tmp,
                    op0=ALU.mult, op1=ALU.add)
                if tt == 3 or tt == RT - 1:
                    csl = slice(0, 4) if tt == 3 else slice(4, 8)
                    nc.sync.dma_start(
                        out=out_d.ap().rearrange("(t p) n -> p t n", p=128)[
                            :, csl, :],
                        in_=out_sb[:, csl, :],
                    )

    nc.compile()
    return nc


_CACHED = None


def kernel(**inputs):
    global LAST, _CACHED
    inp = {k: np.asarray(v) for k, v in inputs.items()}

    bias_map = {"bq": "bq", "bk": "bk", "ba1": "ba1", "ba2": "ba2",
                "bs1": "bs1", "bs2": "bs2", "bo": "bo"}
    has_bias = {k: bool(np.any(inp[v])) for k, v in bias_map.items()}
    has_bias["bvb"] = bool(np.any(inp["bv"]) or np.any(inp["bb"]))

    key = tuple(sorted(has_bias.items()))
    if _CACHED is None or _CACHED[0] != key:
        _CACHED = (key, _build(has_bias))
    nc = _CACHED[1]

    bf = lambda x: np.ascontiguousarray(x.astype(BF16))
    bfT = lambda x: np.ascontiguousarray(x.astype(BF16).T)
    # Gate pre-activations are halved on the host so sigmoid(x)=0.5*tanh(x/2)+0.5
    # and silu(x)=x*sigmoid(x) reduce to tanh + one scalar_tensor_tensor op.
    # The resulting global factor 2 on v1/attn cancels in rmsnorm; the factor 2
    # from the shortcut gate is folded into Wo (with g_rms).
    wo_fold = 0.5 * inp["g_rms"][:, None] * inp["Wo"]
    weights = {
        "wq": bf(0.5 * inp["Wq"]), "wk": bf(0.5 * inp["Wk"]),
        "wvb": bf(0.5 * np.concatenate([inp["Wv"], inp["Wb"]], axis=1)),
        "wa1": bf(inp["Wa1"]), "ws1": bf(inp["Ws1"]),
        "wa2": bf(0.5 * inp["Wa2"]), "ws2": bf(0.5 * inp["Ws2"]),
        "wo": bf(wo_fold),
    }
    if has_bias["bq"]:
        weights["bq"] = bf(0.5 * inp["bq"][None, :])
    if has_bias["bk"]:
        weights["bk"] = bf(0.5 * inp["bk"][None, :])
    if has_bias["bvb"]:
        weights["bvb"] = bf(0.5 * np.concatenate([inp["bv"], inp["bb"]])[None, :])
    if has_bias["ba1"]:
        weights["ba1"] = bf(inp["ba1"][None, :])
    if has_bias["ba2"]:
        weights["ba2"] = bf(0.5 * inp["ba2"][None, :])
    if has_bias["bs1"]:
        weights["bs1"] = bf(inp["bs1"][None, :])
    if has_bias["bs2"]:
        weights["bs2"] = bf(0.5 * inp["bs2"][None, :])
    if has_bias["bo"]:
        weights["bo"] = bf(inp["bo"][None, :])

    in_maps = []
    for c in range(NCORES):
        b, h = c // 2, c % 2
        m = dict(weights)
        m["hqT"] = bfT(inp["hidden_query"][b, h * R:(h + 1) * R])
        m["hkT"] = bfT(inp["hidden_key"][b])
        m["hvT"] = bfT(inp["hidden_value"][b])
        m["hsT"] = bfT(inp["hidden_shortcut"][b, h * R:(h + 1) * R])
        in_maps.append(m)

    LAST = run_bass_kernel_spmd(nc, in_maps, core_ids=list(range(NCORES)))

    out = np.empty((B, L, D), np.float32)
    for c in range(NCORES):
        b, h = c // 2, c % 2
        out[b, h * R:(h + 1) * R] = LAST.results[c]["out"]
    return out


if __name__ == "__main__":
    rng = np.random.default_rng(0)
    fake = {}
    fake["hidden_query"] = rng.standard_normal((B, L, E), dtype=np.float32)
    fake["hidden_key"] = rng.standard_normal((B, L, E), dtype=np.float32)
    fake["hidden_value"] = rng.standard_normal((B, L, E), dtype=np.float32)
    fake["hidden_shortcut"] = rng.standard_normal((B, L, E), dtype=np.float32)
    for n, s in [("Wq", (E, D)), ("Wk", (E, D)), ("Wv", (E, D)), ("Wa1", (E, 32)),
                 ("Wa2", (32, D)), ("Wb", (E, D)), ("Ws1", (E, 32)), ("Ws2", (32, D)),
                 ("Wo", (D, D))]:
        fake[n] = rng.standard_normal(s, dtype=np.float32) * 0.05
    for n, s in [("bq", D), ("bk", D), ("bv", D), ("ba1", 32), ("ba2", D),
                 ("bb", D), ("bs1", 32), ("bs2", D), ("bo", D)]:
        fake[n] = np.zeros(s, np.float32)
    fake["g_rms"] = np.ones(D, np.float32)
    o = kernel(**fake)
    print("ran:", o.shape, o.dtype, np.abs(o).max())


# revision 16
# speedup vs baseline: 1.6332x; 1.0607x over previous
"""Trainium2 Bass kernel for gated single-head attention (B=4, L=2048, E=512, D=64).

Sharding: data-parallel over 8 cores; core c handles batch b=c//2, query-row
half h=c%2 (1024 query rows). hk/hv are processed per-core for the full batch.

Math restructuring (validated in numpy against the jax reference):
  - q,k are L2-normalized so scores s = (q^.k^)/8 lie in [-1/8, 1/8]; softmax
    exp is linearized: e = 1 + s (rel err 6e-6 after rmsnorm, which cancels
    the near-uniform quadratic term). The attention then COLLAPSES to a
    64x64 bilinear form:
        attn[i] = P0 + rs_q_i * (G^T q_i),
        G = sum_j (k_j/|k_j|) (x) v1_j   [64x64],  P0 = sum_j v1_j.
    No 2048x1024 score matrix, no exp, no per-score evacuation.
  - rs_q (1/(8|q_i|)) and the rmsnorm scale are per-query; both are applied
    AFTER the final Wo projection (queries land on partitions there), using
    rmsnorm(P0 + r*P1) algebra: ms*64 = c0 + 2*r*u + r^2*w with
    c0 = |P0|^2, u = P0.P1_i, w = |P1_i|^2 - all computed by tiny matmuls.
  - inputs are pre-transposed AND pre-cast to bf16 on the host: zero PE
    transposes for the projections, and half the HBM traffic.
  - the query path is stacked two 512-query halves on 128 partitions
    (G / P0 / wo duplicated into partitions 64:128, matmuls use quadrant
    tile positions) so every elementwise op runs at full 128-lane rate.
  - gates go through Tanh (sigma(x) = tanh(x/2)/2 + 1/2, the 1/2s folded
    into host-side weights); in-phase rsqrts are DVE/Pool bit-trick Newton;
    the tail switches the ACT table once (dummy Sqrt) and uses
    Sqrt + vector.reciprocal.
Compute dtype bf16, accumulation f32 in PSUM.
"""

import os
import sys

import numpy as np

try:
    import concourse.bass as bass
except ImportError:  # staged container path
    sys.path.insert(0, "/opt/trn_rl_repo")
    import concourse.bass as bass

import ml_dtypes
from contextlib import ExitStack

import concourse.bacc as bacc
import concourse.tile as tile
from concourse import mybir
from concourse.bass_utils import run_bass_kernel_spmd
from concourse.masks import make_identity

BF16 = ml_dtypes.bfloat16
F32 = mybir.dt.float32
BF = mybir.dt.bfloat16
AF = mybir.ActivationFunctionType
ALU = mybir.AluOpType

B, L, E, D = 4, 2048, 512, 64
NCORES = 8
R = L // 2          # 1024 query rows per core
RT = R // 128       # 8 query m-tiles per core
KT = L // 128       # 16 kv m-tiles per core
EC = E // 128       # 4 contraction chunks
EPS_RMS = 4e-6      # 1e-6 * 4 (v1 carries a global factor 2)

LAST = None  # BassKernelResults of the most recent run (for test harness)


def _build(has_bias):
    """Build the per-core SPMD program. has_bias: dict of bool flags."""
    nc = bacc.Bacc(
        "TRN2",
        target_bir_lowering=False,
        debug=False,
        enable_asserts=False,
        num_devices=NCORES,
    )

    hqT_d = nc.dram_tensor("hqT", [E, R], BF, kind="ExternalInput")
    hkT_d = nc.dram_tensor("hkT", [E, L], BF, kind="ExternalInput")
    hvT_d = nc.dram_tensor("hvT", [E, L], BF, kind="ExternalInput")
    hsT_d = nc.dram_tensor("hsT", [E, R], BF, kind="ExternalInput")
    wq_d = nc.dram_tensor("wq", [E, D], BF, kind="ExternalInput")
    wk_d = nc.dram_tensor("wk", [E, D], BF, kind="ExternalInput")
    wvb_d = nc.dram_tensor("wvb", [E, 2 * D], BF, kind="ExternalInput")
    wa1_d = nc.dram_tensor("wa1", [E, 32], BF, kind="ExternalInput")
    ws1_d = nc.dram_tensor("ws1", [E, 32], BF, kind="ExternalInput")
    wa2_d = nc.dram_tensor("wa2", [32, D], BF, kind="ExternalInput")
    ws2_d = nc.dram_tensor("ws2", [32, D], BF, kind="ExternalInput")
    wo_d = nc.dram_tensor("wo", [D, D], BF, kind="ExternalInput")
    bias_d = {}
    for name, n in [("bq", D), ("bk", D), ("bvb", 2 * D), ("ba1", 32),
                    ("ba2", D), ("bs1", 32), ("bs2", D), ("bo", D)]:
        if has_bias[name]:
            bias_d[name] = nc.dram_tensor(name, [1, n], BF, kind="ExternalInput")
    out_d = nc.dram_tensor("out", [R, D], F32, kind="ExternalOutput")

    with tile.TileContext(nc) as tc, ExitStack() as ctx:
        consts = ctx.enter_context(tc.tile_pool(name="consts", bufs=1))
        persist = ctx.enter_context(tc.tile_pool(name="persist", bufs=1))

        ones128c = consts.tile([128, 1], BF)
        nc.vector.memset(ones128c, 1.0)
        ones64x128 = consts.tile([64, 128], BF)
        nc.vector.memset(ones64x128, 1.0)
        onef = consts.tile([1, 1], F32)
        nc.vector.memset(onef, 1.0)
        eps128 = consts.tile([128, 1], F32)
        nc.vector.memset(eps128, EPS_RMS)
        ident64 = consts.tile([64, 64], BF)
        make_identity(nc, ident64)
        magic_i = consts.tile([128, KT], mybir.dt.int32)
        nc.vector.memset(magic_i, 0x5F3759DF)
        any_bias = any(has_bias.values())
        if any_bias:
            ones_row = consts.tile([1, 512], BF)
            nc.vector.memset(ones_row, 1.0)

        # --- weights: kv-path weights early on sync; the rest on scalar ---
        def load_w(d, n, nm, eng):
            t = consts.tile([128, EC, n], BF, name=nm)
            eng.dma_start(out=t, in_=d.ap().rearrange("(c p) n -> p c n", p=128))
            return t

        wk = load_w(wk_d, D, "wk_sb", nc.sync)
        wvb = load_w(wvb_d, 2 * D, "wvb_sb", nc.sync)
        wa1 = load_w(wa1_d, 32, "wa1_sb", nc.scalar)
        wa2 = consts.tile([32, D], BF)
        nc.scalar.dma_start(out=wa2, in_=wa2_d.ap())
        ws1 = load_w(ws1_d, 32, "ws1_sb", nc.scalar)
        ws2 = consts.tile([32, D], BF)
        nc.scalar.dma_start(out=ws2, in_=ws2_d.ap())
        wq = load_w(wq_d, D, "wq_sb", nc.scalar)
        wo2 = consts.tile([128, D], BF)   # wo duplicated into both halves
        nc.scalar.dma_start(out=wo2[0:64, :], in_=wo_d.ap())
        nc.scalar.dma_start(out=wo2[64:128, :], in_=wo_d.ap())
        bias_sb = {}
        for name, t in bias_d.items():
            n = t.shape[1]
            bt = consts.tile([1, n], BF, name=f"{name}_sb")
            nc.scalar.dma_start(out=bt, in_=t.ap())
            bias_sb[name] = bt

        def bias_mm(psum, name):
            """Add per-column bias b[1, n] to psum accumulation via K=1 matmul."""
            if name not in bias_sb:
                return False
            nc.tensor.matmul(psum, ones_row[:, : psum.shape[0]], bias_sb[name],
                             start=False, stop=True)
            return True

        def biasT_mm(psum, name):
            """Add per-row bias (transposed layouts): psum[r, m] += b[r]."""
            if name not in bias_sb:
                return False
            nc.tensor.matmul(psum, bias_sb[name], ones_row[:, : psum.free_size()],
                             start=False, stop=True)
            return True

        # persistent SBUF tensors
        k2o = persist.tile([128, KT, D + 1], BF)    # k/|k| plus a ones column
        nc.vector.memset(k2o[:, :, D:D + 1], 1.0)
        v1 = persist.tile([128, KT, D], BF)
        ss_k = persist.tile([128, KT], F32)
        rs_k = persist.tile([128, KT], F32)
        hq_sb = persist.tile([128, EC, R], BF)
        hs_sb = persist.tile([128, EC, R], BF)
        # query-half-stacked tensors: rows 0:64 queries 0:512, 64:128 rest
        qT_sb = persist.tile([128, 512], BF)
        sqq_sb = persist.tile([128, 512], BF)
        tsc = persist.tile([128, 512], BF)
        P1sb = persist.tile([128, 512], BF)
        sqP1 = persist.tile([128, 512], BF)
        yTA = persist.tile([128, 512], BF)
        yTB = persist.tile([128, 512], BF)
        Gfull = persist.tile([128, D], BF)          # G in rows 0:64 AND 64:128
        P0row = persist.tile([1, D], BF)
        P0col_b = persist.tile([128, 1], BF)        # P0 dup'd in both halves
        P0col_f = persist.tile([128, 1], F32)
        sqP0 = persist.tile([128, 1], BF)
        c0_c = persist.tile([128, 1], F32)
        ssq_c = persist.tile([128, RT], F32)
        rq_c = persist.tile([128, RT], F32)
        rms_c = persist.tile([128, RT], F32)
        rmsq_c = persist.tile([128, RT], F32)
        out_sb = persist.tile([128, RT, D], F32)

        def rsqrt_newton(eng, dst, src, pool, iters=1):
            """dst = 1/sqrt(src) via Quake bit-trick + Newton on `eng`.
            src: [128, n] f32, n <= KT."""
            n = src.shape[-1]
            I32 = mybir.dt.int32
            i1 = pool.tile([128, KT], I32, tag="rqi", name="rqi")[:, :n]
            eng.tensor_scalar(out=i1, in0=src.bitcast(I32), scalar1=1,
                              scalar2=None, op0=ALU.arith_shift_right)
            x0 = pool.tile([128, KT], F32, tag="rqx", name="rqx")[:, :n]
            eng.tensor_tensor(out=x0.bitcast(I32), in0=magic_i[:, :n],
                              in1=i1, op=ALU.subtract)
            h = pool.tile([128, KT], F32, tag="rqh", name="rqh")[:, :n]
            eng.tensor_scalar_mul(h, src, 0.5)
            cur = x0
            for it in range(iters):
                t = pool.tile([128, KT], F32, tag="rqt", name="rqt")[:, :n]
                eng.tensor_mul(t, cur, cur)
                eng.tensor_mul(t, t, h)
                eng.tensor_scalar(out=t, in0=t, scalar1=-1.0,
                                  scalar2=1.5, op0=ALU.mult, op1=ALU.add)
                dst_it = dst if it == iters - 1 else pool.tile(
                    [128, KT], F32, tag="rqn", name="rqn")[:, :n]
                eng.tensor_mul(dst_it, cur, t)
                cur = dst_it

        with tc.tile_pool(name="loadk", bufs=4) as loadk, \
             tc.tile_pool(name="loadv", bufs=4) as loadv, \
             tc.tile_pool(name="sig", bufs=4) as sig, \
             tc.tile_pool(name="psA", bufs=3, space="PSUM") as psA, \
             tc.tile_pool(name="psP1", bufs=1, space="PSUM") as psP1, \
             tc.tile_pool(name="psG", bufs=1, space="PSUM") as psG, \
             tc.tile_pool(name="psC", bufs=1, space="PSUM") as psC, \
             tc.tile_pool(name="psPo", bufs=2, space="PSUM") as psPo:

            G_ps = psG.tile([128, D], F32, name="G_ps")
            pcols = psC.tile([128, RT, 3], F32, tag="pc", name="pcols")

            # ================= k/v phase: 4 blocks of 512 keys =================
            hkT_src = hkT_d.ap().rearrange("(c p) r -> p c r", p=128)
            hvT_src = hvT_d.ap().rearrange("(c p) r -> p c r", p=128)
            for blk in range(4):
                ks = slice(blk * 512, (blk + 1) * 512)
                hkb = loadk.tile([128, EC, 512], BF, tag="hk", name="hkb")
                nc.gpsimd.dma_start(out=hkb, in_=hkT_src[:, :, ks])
                hvb = loadv.tile([128, EC, 512], BF, tag="hv", name="hvb")
                nc.sync.dma_start(out=hvb, in_=hvT_src[:, :, ks])
                if blk == 0:
                    nc.sync.dma_start(
                        out=hq_sb,
                        in_=hqT_d.ap().rearrange("(c p) r -> p c r", p=128))
                    nc.sync.dma_start(
                        out=hs_sb,
                        in_=hsT_d.ap().rearrange("(c p) r -> p c r", p=128))

                # ---- k projection (row-major) + silu + |k| ----
                pk = psA.tile([128, 4, D], F32, tag="proj", name="pk")
                for t in range(4):
                    for c in range(EC):
                        nc.tensor.matmul(
                            pk[:, t, :], hkb[:, c, t * 128:(t + 1) * 128],
                            wk[:, c, :], start=(c == 0),
                            stop=(c == EC - 1 and not has_bias["bk"]))
                    bias_mm(pk[:, t, :], "bk")
                kf = sig.tile([128, 4, D], BF, tag="kf", name="kf")
                nc.scalar.activation(kf, pk, AF.Silu, scale=2.0)
                ksq = sig.tile([128, 4, D], BF, tag="ksq", name="ksq")
                nc.scalar.activation(ksq, kf, AF.Square)
                g = blk * 4
                nc.vector.reduce_sum(
                    ss_k[:, g:g + 4].rearrange("p (a b) -> p a b", b=1),
                    ksq, axis=mybir.AxisListType.X)
                rsqrt_newton(nc.vector, rs_k[:, g:g + 4], ss_k[:, g:g + 4],
                             sig, iters=1)
                rsb = rs_k[:, g:g + 4].rearrange("p (a b) -> p a b", b=1)
                kf_b, rs_b = bass.broadcast_tensor_aps(kf, rsb)
                nc.vector.tensor_tensor(out=k2o[:, g:g + 4, :D], in0=kf_b,
                                        in1=rs_b, op=ALU.mult)

                # ---- v | beta projection + silu ----
                vbt = sig.tile([128, 4, 2 * D], BF, tag="vbt", name="vbt")
                vf = sig.tile([128, 4, D], BF, tag="vf", name="vf")
                for u in range(2):
                    pvb = psA.tile([128, 2, 2 * D], F32, tag="proj", name="pvb")
                    for hh in range(2):
                        t = 2 * u + hh
                        for c in range(EC):
                            nc.tensor.matmul(
                                pvb[:, hh, :],
                                hvb[:, c, t * 128:(t + 1) * 128],
                                wvb[:, c, :], start=(c == 0),
                                stop=(c == EC - 1 and not has_bias["bvb"]))
                        bias_mm(pvb[:, hh, :], "bvb")
                    nc.scalar.activation(vbt[:, 2 * u:2 * u + 2, D:], pvb[:, :, D:],
                                         AF.Tanh)
                    nc.scalar.activation(vf[:, 2 * u:2 * u + 2, :],
                                         pvb[:, :, :D], AF.Silu, scale=2.0)

                # ---- alpha: a1T (weight-stationary) then a2 (row-major) ----
                pa1 = psA.tile([32, 512], F32, tag="proj", name="pa1")
                for c in range(EC):
                    nc.tensor.matmul(pa1, wa1[:, c, :], hvb[:, c, :],
                                     start=(c == 0),
                                     stop=(c == EC - 1 and not has_bias["ba1"]))
                biasT_mm(pa1, "ba1")
                a1T = sig.tile([32, 512], BF, tag="a1T", name="a1T")
                nc.vector.tensor_copy(a1T, pa1)
                pa2 = psA.tile([128, 4, D], F32, tag="proj", name="pa2")
                for t in range(4):
                    nc.tensor.matmul(pa2[:, t, :],
                                     a1T[:, t * 128:(t + 1) * 128], wa2,
                                     start=True, stop=not has_bias["ba2"])
                    bias_mm(pa2[:, t, :], "ba2")
                alf = sig.tile([128, 4, D], BF, tag="sig", name="alf")
                nc.scalar.activation(alf, pa2, AF.Tanh)
                # v1 = vf*(alf+1) + (vbt_beta+1)   (= 2*(v*alpha+beta))
                t1 = sig.tile([128, 4, D], BF, tag="t1", name="t1")
                nc.vector.scalar_tensor_tensor(
                    out=t1, in0=alf, scalar=1.0, in1=vf,
                    op0=ALU.add, op1=ALU.mult)
                nc.vector.scalar_tensor_tensor(
                    out=v1[:, g:g + 4, :], in0=vbt[:, :, D:], scalar=1.0,
                    in1=t1, op0=ALU.add, op1=ALU.add)

                # ---- G accumulation: G[0:64] += k2^T v1, G[64] += sum v1 ----
                for t in range(4):
                    jt = g + t
                    nc.tensor.matmul(G_ps[0:65, :], k2o[:, jt, :],
                                     v1[:, jt, :], start=(jt == 0),
                                     stop=(jt == KT - 1))

                if blk == 1:
                    # ======= query path (overlaps kv blocks 2-3) =======
                    pq = psA.tile([128, 512], F32, tag="proj", name="pq")
                    for h in range(2):
                        for c in range(EC):
                            nc.tensor.matmul(
                                pq[64 * h:64 * h + 64, :], wq[:, c, :],
                                hq_sb[:, c, h * 512:(h + 1) * 512],
                                start=(c == 0),
                                stop=(c == EC - 1 and not has_bias["bq"]))
                        biasT_mm(pq[64 * h:64 * h + 64, :], "bq")
                    nc.scalar.activation(qT_sb, pq, AF.Silu, scale=2.0)
                    nc.scalar.activation(sqq_sb, qT_sb, AF.Square)
                    for tt in range(RT):
                        h, cc = tt // 4, tt % 4
                        hp = slice(64 * h, 64 * h + 64)
                        cs = slice(cc * 128, (cc + 1) * 128)
                        nc.tensor.matmul(pcols[:, tt, 0:1], sqq_sb[hp, cs],
                                         ones128c[hp, :], start=True, stop=True)
                    nc.vector.tensor_copy(ssq_c, pcols[:, :, 0])
                    ssq64 = sig.tile([128, RT], F32, tag="ep", name="ssq64")
                    nc.vector.tensor_scalar_mul(ssq64, ssq_c, 64.0)
                    rsqrt_newton(nc.vector, rq_c, ssq64, sig, iters=1)

                    # ======= shortcut path =======
                    ps2 = psA.tile([128, 512], F32, tag="proj", name="ps2")
                    for h in range(2):
                        ps1 = psPo.tile([32, 512], F32, tag="po", name="ps1")
                        for c in range(EC):
                            nc.tensor.matmul(
                                ps1, ws1[:, c, :],
                                hs_sb[:, c, h * 512:(h + 1) * 512],
                                start=(c == 0),
                                stop=(c == EC - 1 and not has_bias["bs1"]))
                        biasT_mm(ps1, "bs1")
                        s1T = sig.tile([32, 512], BF, tag="a1T", name="s1T")
                        nc.vector.tensor_copy(s1T, ps1)
                        nc.tensor.matmul(ps2[64 * h:64 * h + 64, :], ws2, s1T,
                                         start=True, stop=not has_bias["bs2"])
                        biasT_mm(ps2[64 * h:64 * h + 64, :], "bs2")
                    nc.scalar.activation(tsc, ps2, AF.Tanh)

            # ================= tail =================
            # early ACT table switch (tanh set -> sqrt set) behind PE work
            sqd = sig.tile([1, 1], F32, tag="sqd", name="sqd")
            nc.scalar.activation(sqd, onef, AF.Sqrt)

            # G / P0 evacuation + duplication into partitions 64:128
            nc.vector.tensor_copy(Gfull[0:64, :], G_ps[0:64, :])
            nc.vector.tensor_copy(P0row, G_ps[64:65, :])
            gd_ps = psPo.tile([128, D], F32, tag="po", name="gd_ps")
            nc.tensor.matmul(gd_ps[64:128, :], ident64, Gfull[0:64, :],
                             start=True, stop=True)
            nc.vector.tensor_copy(Gfull[64:128, :], gd_ps[64:128, :])
            p0c_ps = psPo.tile([128, 1], BF, tag="po", name="p0c_ps")
            nc.tensor.transpose(p0c_ps[0:64, :], P0row, ones128c[0:1, :])
            nc.tensor.transpose(p0c_ps[64:128, :], P0row, ones128c[0:1, :])
            nc.vector.tensor_copy(P0col_b, p0c_ps)
            nc.vector.tensor_copy(P0col_f, p0c_ps)
            nc.scalar.activation(sqP0, P0col_b, AF.Square)
            c0_ps = psPo.tile([128, 1], F32, tag="po", name="c0_ps")
            nc.tensor.matmul(c0_ps, ones64x128, sqP0[0:64, :],
                             start=True, stop=True)
            nc.vector.tensor_copy(c0_c, c0_ps)

            # P1 = G^T qT (both halves via quadrants)
            pP1 = psP1.tile([128, 512], F32, tag="p1", name="pP1")
            nc.tensor.matmul(pP1[0:64, :], Gfull[0:64, :], qT_sb[0:64, :],
                             start=True, stop=True)
            nc.tensor.matmul(pP1[64:128, :], Gfull[64:128, :], qT_sb[64:128, :],
                             start=True, stop=True)
            # yTB = (tsc+1) * P1 ; yTA = (tsc+1) * P0 = tsc*P0 + P0
            nc.vector.scalar_tensor_tensor(
                out=yTB, in0=tsc, scalar=1.0, in1=pP1,
                op0=ALU.add, op1=ALU.mult)
            nc.scalar.activation(yTA, tsc, AF.Identity,
                                 scale=P0col_f, bias=P0col_f)
            nc.scalar.activation(P1sb, pP1, AF.Copy)
            nc.scalar.activation(sqP1, pP1, AF.Square)

            # u / w columns per query tile
            for tt in range(RT):
                h, cc = tt // 4, tt % 4
                hp = slice(64 * h, 64 * h + 64)
                cs = slice(cc * 128, (cc + 1) * 128)
                nc.tensor.matmul(pcols[:, tt, 1:2], P1sb[hp, cs],
                                 P0col_b[hp, :], start=True, stop=True)
                nc.tensor.matmul(pcols[:, tt, 2:3], sqP1[hp, cs],
                                 ones128c[hp, :], start=True, stop=True)
            uw = sig.tile([128, RT, 2], F32, tag="uw", name="uw")
            nc.vector.tensor_copy(uw, pcols[:, :, 1:3])

            # ms*64 = c0 + 2*rq*u + rq^2*w ; rms = rsqrt(ms + eps)
            tA = sig.tile([128, RT], F32, tag="ep", name="tA")
            nc.vector.tensor_mul(tA, rq_c, uw[:, :, 0])
            tB = sig.tile([128, RT], F32, tag="ep2", name="tB")
            nc.vector.tensor_mul(tB, rq_c, rq_c)
            nc.vector.tensor_mul(tB, tB, uw[:, :, 1])
            nc.vector.scalar_tensor_tensor(
                out=tB, in0=tA, scalar=2.0, in1=tB,
                op0=ALU.mult, op1=ALU.add)
            nc.vector.tensor_scalar(out=tB, in0=tB, scalar1=c0_c,
                                    scalar2=None, op0=ALU.add)
            srt = sig.tile([128, RT], F32, tag="ep3", name="srt")
            nc.scalar.activation(srt, tB, AF.Sqrt, scale=1.0 / 64, bias=eps128)
            nc.vector.reciprocal(rms_c, srt)
            nc.vector.tensor_mul(rmsq_c, rms_c, rq_c)

            # final Wo projections + per-query scaling
            for grp in range(2):
                po4 = psPo.tile([128, 4, 2, D], F32, tag="po", name="po4")
                for j in range(4):
                    tt = grp * 4 + j
                    h, cc = tt // 4, tt % 4
                    hp = slice(64 * h, 64 * h + 64)
                    cs = slice(cc * 128, (cc + 1) * 128)
                    nc.tensor.matmul(po4[:, j, 0, :], yTA[hp, cs], wo2[hp, :],
                                     start=True, stop=not has_bias["bo"])
                    bias_mm(po4[:, j, 0, :], "bo")
                    nc.tensor.matmul(po4[:, j, 1, :], yTB[hp, cs], wo2[hp, :],
                                     start=True, stop=True)
                for j in range(4):
                    tt = grp * 4 + j
                    tmp = sig.tile([128, D], F32, tag="tmp", name="tmp")
                    nc.scalar.activation(tmp, po4[:, j, 0, :], AF.Copy,
                                         scale=rms_c[:, tt:tt + 1])
                    nc.vector.scalar_tensor_tensor(
                        out=out_sb[:, tt, :], in0=po4[:, j, 1, :],
                        scalar=rmsq_c[:, tt:tt + 1], in1=tmp,
                        op0=ALU.mult, op1=ALU.add)
                csl = slice(grp * 4, grp * 4 + 4)
                nc.sync.dma_start(
                    out=out_d.ap().rearrange("(t p) n -> p t n", p=128)[
                        :, csl, :],
                    in_=out_sb[:, csl, :],
                )

    nc.compile()
    return nc


_CACHED = None


def kernel(**inputs):
    global LAST, _CACHED
    inp = {k: np.asarray(v) for k, v in inputs.items()}

    bias_map = {"bq": "bq", "bk": "bk", "ba1": "ba1", "ba2": "ba2",
                "bs1": "bs1", "bs2": "bs2", "bo": "bo"}
    has_bias = {k: bool(np.any(inp[v])) for k, v in bias_map.items()}
    has_bias["bvb"] = bool(np.any(inp["bv"]) or np.any(inp["bb"]))

    key = tuple(sorted(has_bias.items()))
    if _CACHED is None or _CACHED[0] != key:
        _CACHED = (key, _build(has_bias))
    nc = _CACHED[1]

    bf = lambda x: np.ascontiguousarray(x.astype(BF16))
    bfT = lambda x: np.ascontiguousarray(x.astype(BF16).T)
    # Gate pre-activations are halved on the host so sigmoid(x)=0.5*tanh(x/2)+0.5
    # and silu(x)=x*sigmoid(x) reduce to tanh + one scalar_tensor_tensor op.
    # The resulting global factor 2 on v1/attn cancels in rmsnorm; the factor 2
    # from the shortcut gate is folded into Wo (with g_rms).
    wo_fold = 0.5 * inp["g_rms"][:, None] * inp["Wo"]
    weights = {
        "wq": bf(0.5 * inp["Wq"]), "wk": bf(0.5 * inp["Wk"]),
        "wvb": bf(0.5 * np.concatenate([inp["Wv"], inp["Wb"]], axis=1)),
        "wa1": bf(inp["Wa1"]), "ws1": bf(inp["Ws1"]),
        "wa2": bf(0.5 * inp["Wa2"]), "ws2": bf(0.5 * inp["Ws2"]),
        "wo": bf(wo_fold),
    }
    if has_bias["bq"]:
        weights["bq"] = bf(0.5 * inp["bq"][None, :])
    if has_bias["bk"]:
        weights["bk"] = bf(0.5 * inp["bk"][None, :])
    if has_bias["bvb"]:
        weights["bvb"] = bf(0.5 * np.concatenate([inp["bv"], inp["bb"]])[None, :])
    if has_bias["ba1"]:
        weights["ba1"] = bf(inp["ba1"][None, :])
    if has_bias["ba2"]:
        weights["ba2"] = bf(0.5 * inp["ba2"][None, :])
    if has_bias["bs1"]:
        weights["bs1"] = bf(inp["bs1"][None, :])
    if has_bias["bs2"]:
        weights["bs2"] = bf(0.5 * inp["bs2"][None, :])
    if has_bias["bo"]:
        weights["bo"] = bf(inp["bo"][None, :])

    in_maps = []
    for c in range(NCORES):
        b, h = c // 2, c % 2
        m = dict(weights)
        m["hqT"] = bfT(inp["hidden_query"][b, h * R:(h + 1) * R])
        m["hkT"] = bfT(inp["hidden_key"][b])
        m["hvT"] = bfT(inp["hidden_value"][b])
        m["hsT"] = bfT(inp["hidden_shortcut"][b, h * R:(h + 1) * R])
        in_maps.append(m)

    LAST = run_bass_kernel_spmd(nc, in_maps, core_ids=list(range(NCORES)))

    out = np.empty((B, L, D), np.float32)
    for c in range(NCORES):
        b, h = c // 2, c % 2
        out[b, h * R:(h + 1) * R] = LAST.results[c]["out"]
    return out


if __name__ == "__main__":
    rng = np.random.default_rng(0)
    fake = {}
    fake["hidden_query"] = rng.standard_normal((B, L, E), dtype=np.float32)
    fake["hidden_key"] = rng.standard_normal((B, L, E), dtype=np.float32)
    fake["hidden_value"] = rng.standard_normal((B, L, E), dtype=np.float32)
    fake["hidden_shortcut"] = rng.standard_normal((B, L, E), dtype=np.float32)
    for n, s in [("Wq", (E, D)), ("Wk", (E, D)), ("Wv", (E, D)), ("Wa1", (E, 32)),
                 ("Wa2", (32, D)), ("Wb", (E, D)), ("Ws1", (E, 32)), ("Ws2", (32, D)),
                 ("Wo", (D, D))]:
        fake[n] = rng.standard_normal(s, dtype=np.float32) * 0.05
    for n, s in [("bq", D), ("bk", D), ("bv", D), ("ba1", 32), ("ba2", D),
                 ("bb", D), ("bs1", 32), ("bs2", D), ("bo", D)]:
        fake[n] = np.zeros(s, np.float32)
    fake["g_rms"] = np.ones(D, np.float32)
    o = kernel(**fake)
    print("ran:", o.shape, o.dtype, np.abs(o).max())
